# revision 25
# baseline (speedup 1.0000x reference)
"""Trainium2 Bass kernel for CHMSA (cross-covariance multi-head self-attention
with a ConvNorm qkv stem).

Problem (hardcoded):
  x         [16, 64, 64, 256] f32
  dw_kernel [3, 3, 1, 256]    depthwise 3x3, SAME
  bn_gamma/bn_beta [256]      per-channel affine after dwconv
  pw_kernel [256, 768]        1x1 conv -> qkv
  q_bias/v_bias [256]         qkv bias = concat([q_bias, 0, v_bias])
  scale     [8,1,1]           per-head logit scale, s = exp(min(scale, ln 100))
  proj_w    [256, 256], proj_b [256]

Sharding: pure data-parallel over batch: 16 images / 8 cores = 2 images/core.
No collectives.

Everything upstream of PSUM runs in fp16 (e5m10): fp16's 10-bit mantissa
keeps the end-to-end rel err at ~2.8e-3 (bf16's 7-bit mantissa blows the
2e-2 budget through the weight tensors), while every stationary operand
gets the 2-byte half-cost LDWEIGHTS and x DMA traffic halves. All PSUM
accumulation stays f32.

Per-core dataflow (per image, N = 4096 tokens, C = 256):
  1. x is pre-transposed to channel-major [C, N] and cast to fp16 on the
     HOST (make_in_maps); the xt tile is UNPADDED so each x row-block DMA
     is one contiguous 1KB-per-partition run (a zero-halo layout produced
     128-byte bursts that starved the dwconv at startup).
  2. dwconv: 9 diagonal fp16 matmuls per PSUM tile (channel-major), gamma
     folded into the diagonal weights. SAME padding via per-tap AP
     clipping: edge psum rows/cols simply don't receive out-of-image tap
     contributions. The two channel chunks are emitted as separate halves
     so qk fronts can interleave between them on the PE queue.
  3. qkv: q,k token-major per token-chunk PAIR: ACT square -> GpSimd
     half-fold -> DVE grouped reduce -> rsqrt per half (1/|q|, 1/|k|) ->
     ONE DVE op per chunk scales the whole 512-wide psum tile into the
     fp16 qskb tile (q-half by 1/|q|, k-half by 1/|k|; the per-head scale
     s is folded into the softmax logit gather instead); v channel-major.
  4. attn gram: fp16 [128,128] matmuls accumulated over all 32 token
     chunks. BOTH head-groups accumulate concurrently in one PSUM bank
     (att is [128, 256] = 1KB of the 2KB bank; start_tensor_calc zeroes
     the whole bank, so only the very first matmul starts and only the
     very last stops). Softmax applies s_h during the DVE logit gather;
     32x32 DVE transposes build attn^T (fp16).
  5. out_cm = attn^T-weighted v (channel-major, fp16), proj back to
     token-major (fp16 weights), DMA out. The previous image's C blocks
     interleave into the next image's phase between that stage's qk
     front and gram so the PE never stalls on the previous softmax, and
     the next image's first x row-blocks prefetch during this image's
     late stages (region-level dependency tracking permits it).
"""

import math

import numpy as np
import ml_dtypes

import concourse.bass as bass
import concourse.mybir as mybir
import concourse.tile as tile
from concourse import bacc
from concourse.bass_utils import run_bass_kernel_spmd

F32 = mybir.dt.float32
F32R = mybir.dt.float32r
F16 = mybir.dt.float16
BF16 = mybir.dt.bfloat16
AF = mybir.ActivationFunctionType
ALU = mybir.AluOpType

B, H, W, C = 16, 64, 64, 256
N = H * W              # 4096 tokens per image
HEADS = 8
HD = C // HEADS        # 32
NCORES = 8
IMGS = B // NCORES     # 2 images per core
NCH = C // 128         # 2 channel chunks
LOG_MAX_SCALE = float(np.log(100.0))

# dwconv tap offsets (dh, dw), center first so it can carry start=True with
# full-tile coverage; the ragged edge taps then accumulate.
TAPS = [(0, 0), (-1, -1), (-1, 0), (-1, 1), (0, -1), (0, 1), (1, -1), (1, 0), (1, 1)]

HBLK = 8               # h-rows per dwconv psum tile -> free dim 8*64 = 512
NBLK = N // 128        # 32 token chunks of 128

# ---- engine assignment knobs ----
# (GpSimd cannot read PSUM, so all PSUM evictions live on ACT/DVE.)
VT_EVICT_DVE = False    # DVE queue feeds the qk chain; keep v eviction on ACT


def _build_program(consts, add_qbias, add_pbias, reps=1):
    nc = bacc.Bacc()

    # x arrives channel-major ([IMGS, C, N], transposed + cast to fp16 on
    # the HOST) so the dwconv input tile loads directly with no PE
    # transposes; fp16 halves the x DMA traffic and makes every stationary
    # weight load a 2-byte (half-cost) LDWEIGHTS.
    x_dr = nc.dram_tensor("x", [IMGS, C, N], F16, kind="ExternalInput")
    out_dr = nc.dram_tensor("out", [IMGS, N, C], F32, kind="ExternalOutput")

    diag_dr = nc.inline_tensor(consts["diag"], "cdiag")        # [128, NCH, 9, 128]
    pwqk_dr = nc.inline_tensor(consts["pwqk"], "cpwqk")        # [128, NCH, 512]
    pwv_dr = nc.inline_tensor(consts["pwv"], "cpwv")           # [128, NCH, NCH, 128]
    projw_dr = nc.inline_tensor(consts["projw"], "cprojw")     # [128, NCH, 256] bf16
    beta_dr = nc.inline_tensor(consts["beta"], "cbeta")        # [128, NCH]
    vb_dr = nc.inline_tensor(consts["vb"], "cvb")              # [128, NCH]
    s_host = [float(v) for v in consts["s_host"]]              # python floats
    if add_qbias:
        qb_dr = nc.inline_tensor(consts["qb"], "cqb")          # [128, 256]
    if add_pbias:
        pb_dr = nc.inline_tensor(consts["pb"], "cpb")          # [128, 256]

    use_beta = bool(np.any(consts["beta"]))
    use_vb = bool(np.any(consts["vb"]))

    with tile.TileContext(nc) as tc:
        with (
            tc.tile_pool(name="singles", bufs=1) as singles,
            tc.tile_pool(name="xt", bufs=1) as xt_pool,
            tc.tile_pool(name="img_big", bufs=1) as img_pool,
            tc.tile_pool(name="sq", bufs=4) as sq_pool,
            tc.tile_pool(name="wp", bufs=4) as wp_pool,
            tc.tile_pool(name="small", bufs=3) as small,
            tc.tile_pool(name="ostage", bufs=6) as ostage,
            tc.tile_pool(name="ps_mm", bufs=2, space="PSUM") as ps_mm,
            tc.tile_pool(name="ps_qk", bufs=3, space="PSUM") as ps_qk,
            tc.tile_pool(name="ps_c", bufs=2, space="PSUM") as ps_c,
            tc.tile_pool(name="ps_attn", bufs=1, space="PSUM") as ps_attn,
        ):
            # ---- constants into SBUF ----
            # Spread the big const DMAs across engine DGE queues so the
            # first dwconv tile isn't gated on one serial queue: diag's
            # two channel chunks go to the vector and scalar queues (the
            # sync + gpsimd queues carry the first x tiles).
            diag_sb = singles.tile([128, NCH, 9, 128], F16)
            nc.scalar.dma_start(diag_sb[:, 0], diag_dr[:, 0])
            nc.scalar.dma_start(diag_sb[:, 1], diag_dr[:, 1])
            pwqk_sb = singles.tile([128, NCH, 512], F16)
            nc.scalar.dma_start(pwqk_sb[:], pwqk_dr[:])
            pwv_sb = singles.tile([128, NCH, NCH, 128], F16)
            nc.scalar.dma_start(pwv_sb[:], pwv_dr[:])
            projw_sb = singles.tile([128, NCH, 256], F16)
            nc.scalar.dma_start(projw_sb[:], projw_dr[:])
            beta_sb = singles.tile([128, NCH], F32)
            nc.scalar.dma_start(beta_sb[:], beta_dr[:])
            vb_sb = singles.tile([128, NCH], F32)
            nc.scalar.dma_start(vb_sb[:], vb_dr[:])
            if add_qbias:
                qb_sb = singles.tile([128, 256], F32)
                nc.gpsimd.dma_start(qb_sb[:], qb_dr[:])
            if add_pbias:
                pb_sb = singles.tile([128, 256], F32)
                nc.gpsimd.dma_start(pb_sb[:], pb_dr[:])

            # xt is shared by both images (re-DMA'd per image), UNPADDED:
            # SAME-padding is expressed by clipping each tap's matmul APs
            # instead of a zero halo, so every x DMA is a contiguous
            # 1KB-per-partition run (the padded layout produced 128-byte
            # bursts that starved the dwconv at startup).
            xt_sh = xt_pool.tile([128, NCH, H, W], F16,
                                 tag="xt", name="xt_sh")

            def make_img_state(img):
                st = {}
                st["img"] = img
                st["xt"] = xt_sh
                st["yt"] = img_pool.tile([128, NCH, N], F16, tag="yt",
                                         name=f"yt{img}")
                st["vt"] = img_pool.tile([128, NCH, N], F16, tag="vt",
                                         name=f"vt{img}")
                # one PSUM bank holds BOTH head-groups' grams ([128, 256]
                # = 1KB of the 2KB bank). start_tensor_calc zeroes the
                # whole bank, so only the first matmul into the bank may
                # carry start=True and only the very last stop=True.
                st["att"] = ps_attn.tile([128, 2, 128], F32, tag="att",
                                         name=f"att_{img}")
                # [q-half | k-half] per token chunk, already l2-scaled
                st["qskb"] = img_pool.tile([128, NBLK, 512], F16, tag="qskb",
                                           name=f"qskb{img}")
                return st

            def load_rb(img, rb, preload=False):
                # DMA one 8-row block (512 tokens) of channel-major x into
                # the xt tile; the two channel chunks ride separate DGE
                # queues (sync + gpsimd) so they transfer in parallel.
                # Preloads for the NEXT image ride the scalar queue instead:
                # it is idle after the startup constants, while the sync
                # queue carries the previous image's out stores at exactly
                # that point in the schedule.
                for cch in range(NCH):
                    if preload:
                        eng = nc.scalar
                    else:
                        eng = nc.sync if cch == 0 else nc.gpsimd
                    eng.dma_start(
                        xt_sh[:, cch, 8 * rb:8 * (rb + 1), :].rearrange(
                            "p h w -> p (h w)"),
                        x_dr[img, cch * 128:(cch + 1) * 128,
                             rb * 512:(rb + 1) * 512],
                    )

            def dwconv_block(st, hb, cch):
                # SAME padding via AP clipping: each tap's matmul writes only
                # the psum rows/cols whose shifted input lies inside the
                # image; edge cells simply receive fewer tap contributions
                # (they were zeroed by the center tap's start_tensor_calc).
                # One channel chunk per call so the caller can interleave
                # other PE work between the halves.
                h0 = hb * HBLK
                if True:
                    ysl = st["yt"][:, cch, h0 * W:(h0 + HBLK) * W]
                    yp = ps_mm.tile([128, HBLK, W], F32, tag="mm", name="yp")
                    for i, ti in enumerate(range(9)):
                        dh, dw = TAPS[ti]
                        r0 = max(0, -(h0 + dh))
                        r1 = HBLK + min(0, H - (h0 + HBLK + dh))
                        c0 = max(0, -dw)
                        c1 = W - max(0, dw)
                        nc.tensor.matmul(
                            yp[:, r0:r1, c0:c1],
                            diag_sb[:, cch, ti, :],
                            st["xt"][:, cch, h0 + r0 + dh:h0 + r1 + dh,
                                     c0 + dw:c1 + dw],
                            start=(i == 0),
                            stop=(i == 8),
                            skip_group_check=True,
                        )
                    ypf = yp.rearrange("p h w -> p (h w)")
                    if use_beta:
                        nc.scalar.activation(
                            out=ysl, in_=ypf, func=AF.Identity,
                            bias=beta_sb[:, cch:cch + 1],
                        )
                    else:
                        nc.scalar.copy(ysl, ypf)

            def v_block(st, nb):
                for vc in range(NCH):
                    vp = ps_mm.tile([128, 512], F32, tag="mm", name="vp")
                    for kc in range(NCH):
                        nc.tensor.matmul(
                            vp[:],
                            pwv_sb[:, kc, vc, :],
                            st["yt"][:, kc, nb * 512:(nb + 1) * 512],
                            start=(kc == 0),
                            stop=(kc == NCH - 1),
                        )
                    vsl = st["vt"][:, vc, nb * 512:(nb + 1) * 512]
                    if VT_EVICT_DVE:
                        nc.vector.tensor_scalar(
                            out=vsl, in0=vp[:], scalar1=vb_sb[:, vc:vc + 1],
                            scalar2=None, op0=ALU.add,
                        )
                    elif use_vb:
                        nc.scalar.activation(
                            out=vsl, in_=vp[:], func=AF.Identity,
                            bias=vb_sb[:, vc:vc + 1],
                        )
                    else:
                        nc.scalar.copy(vsl, vp[:])

            def qk_front(st, p):
                # two token chunks t0,t1: qkv matmuls -> squares (ACT) ->
                # grouped reduce (GpSimd) -> per-half rsqrt (1/|q|, 1/|k|)
                # -> ONE DVE op per chunk writes the whole bf16 qskb tile
                # straight from PSUM (q-half scaled by 1/|q|, k-half by
                # 1/|k|). The gram matmuls are emitted later (qk_gram) so
                # other PE work covers this vector-side latency.
                qps = []
                sqs = []
                for j in (0, 1):
                    t = 2 * p + j
                    qp = ps_qk.tile([128, 512], F32, tag="qk", name=f"qp{j}")
                    for kc in range(NCH):
                        nc.tensor.matmul(
                            qp[:],
                            st["yt"][:, kc, t * 128:(t + 1) * 128],
                            pwqk_sb[:, kc, :],
                            start=(kc == 0),
                            stop=(kc == NCH - 1),
                        )
                    if add_qbias:
                        nc.vector.tensor_tensor(
                            out=qp[:, 0:256], in0=qp[:, 0:256],
                            in1=qb_sb[:], op=ALU.add,
                        )
                    sq = sq_pool.tile([128, 512], F32, name="sq")
                    nc.scalar.square(sq[:], qp[:])
                    qps.append(qp)
                    sqs.append(sq)
                sqr = wp_pool.tile([128, 2, 16], F32, tag="sqr", name="sqr")
                for j in (0, 1):
                    # single full-width DVE grouped reduce: one less engine
                    # hop in the front chain than the GpSimd-fold variant,
                    # which matters because the chain latency gates the qp
                    # PSUM bank rotation.
                    nc.vector.tensor_reduce(
                        out=sqr[:, j, :],
                        in_=sqs[j].rearrange("p (g d) -> p g d", d=HD),
                        axis=mybir.AxisListType.X,
                        op=ALU.add,
                    )
                # sqr = [|q|^2 (8 heads) | |k|^2 (8 heads)] per chunk;
                # in-place rsqrt gives exactly the per-half scale vector.
                sqf = sqr.rearrange("p a h -> p (a h)")
                nc.vector.reciprocal(sqf, sqf)
                nc.scalar.activation(sqf, sqf, AF.Sqrt)
                for j in (0, 1):
                    t = 2 * p + j
                    nc.vector.tensor_tensor(
                        out=st["qskb"][:, t, :].rearrange(
                            "p (g d) -> p g d", d=HD),
                        in0=qps[j].rearrange("p (g d) -> p g d", d=HD),
                        in1=sqr[:, j, :].unsqueeze(2).broadcast_to(
                            [128, 16, HD]),
                        op=ALU.mult,
                    )

            def qk_gram(st, p):
                # both head-groups accumulate in the shared att bank;
                # start only zeroes once (whole-bank zero), stop only on
                # the very last matmul into the bank.
                for j in (0, 1):
                    t = 2 * p + j
                    for g in range(2):
                        nc.tensor.matmul(
                            st["att"][:, g, :],
                            st["qskb"][:, t, g * 128:(g + 1) * 128],
                            st["qskb"][:, t, 256 + g * 128:256 + (g + 1) * 128],
                            start=(t == 0 and g == 0),
                            stop=(t == NBLK - 1 and g == 1),
                            skip_group_check=True,
                        )

            def softmax_g(st, g):
                if g == 0:
                    st["at_bd"] = small.tile([128, 2, 128], F16, tag="atbd",
                                             name="at_bd")
                at_bd = st["at_bd"]
                asm = small.tile([128, 32], F32, tag="asm", name="asm")
                for j in range(4):
                    h = 4 * g + j
                    nc.vector.tensor_scalar(
                        out=asm[32 * j:32 * j + 32, :],
                        in0=st["att"][32 * j:32 * j + 32, g, 32 * j:32 * j + 32],
                        scalar1=s_host[h], scalar2=None, op0=ALU.mult,
                    )
                mx = small.tile([128, 1], F32, tag="mx", name="mx")
                nc.vector.tensor_reduce(
                    out=mx[:], in_=asm[:], axis=mybir.AxisListType.X,
                    op=ALU.max, negate=True)
                nc.scalar.activation(asm[:], asm[:], AF.Exp, bias=mx[:])
                sm = small.tile([128, 1], F32, tag="sm", name="sm")
                nc.vector.tensor_reduce(
                    out=sm[:], in_=asm[:], axis=mybir.AxisListType.X,
                    op=ALU.add)
                nc.vector.reciprocal(sm[:], sm[:])
                nc.vector.tensor_scalar(
                    out=asm[:], in0=asm[:], scalar1=sm[:], scalar2=None,
                    op0=ALU.mult)
                atf = small.tile([128, 128], F32, tag="atf", name="atf")
                nc.vector.memset(atf[:], 0.0)
                for j in range(4):
                    nc.vector.transpose(
                        atf[32 * j:32 * j + 32, 32 * j:32 * j + 32],
                        asm[32 * j:32 * j + 32, :],
                    )
                nc.vector.tensor_copy(at_bd[:, g, :], atf[:])

            def get_ocm(st):
                img = st["img"]
                if "ocm" not in st:
                    st["ocm"] = img_pool.tile([128, NCH, N], F16, tag="ocm",
                                              name=f"ocm{img}")
                return st["ocm"]

            def c_av(st, nb, g, tail=False):
                # attn^T @ v for one head-group over one 512-token slab; in
                # the trailing loop the tiles borrow the idle ps_qk banks.
                ocm = get_ocm(st)
                op_ = (ps_qk if tail else ps_c).tile(
                    [128, 512], F32, tag="qk" if tail else "cmm",
                    name="op_")
                nc.tensor.matmul(
                    op_[:],
                    st["at_bd"][:, g, :],
                    st["vt"][:, g, nb * 512:(nb + 1) * 512],
                )
                if g == 0:
                    nc.vector.tensor_copy(
                        ocm[:, g, nb * 512:(nb + 1) * 512], op_[:])
                else:
                    nc.scalar.copy(
                        ocm[:, g, nb * 512:(nb + 1) * 512], op_[:])

            def c_proj(st, nb):
                img = st["img"]
                ocm = get_ocm(st)
                for t in range(4 * nb, 4 * nb + 4):
                    pp = ps_c.tile([128, 256], F32, tag="cmm", name="pp")
                    for kc in range(NCH):
                        nc.tensor.matmul(
                            pp[:],
                            ocm[:, kc, t * 128:(t + 1) * 128],
                            projw_sb[:, kc, :],
                            start=(kc == 0),
                            stop=(kc == NCH - 1),
                        )
                    if t % 2 == 0:
                        ot = ostage.tile([128, 2, 256], F32, name="ot")
                        st["ot"] = ot
                    ot = st["ot"]
                    if add_pbias:
                        nc.vector.tensor_tensor(
                            out=ot[:, t % 2, :], in0=pp[:], in1=pb_sb[:],
                            op=ALU.add)
                    elif t % 2 == 0:
                        nc.scalar.copy(ot[:, 0, :], pp[:])
                    else:
                        nc.vector.tensor_copy(ot[:, 1, :], pp[:])
                    if t % 2 == 1:
                        nc.sync.dma_start(
                            out_dr[img, (t - 1) * 128:(t + 1) * 128,
                                   :].rearrange("(g p) c -> p g c", p=128),
                            ot[:],
                        )

            def c_block(st, nb):
                c_av(st, nb, 0)
                c_av(st, nb, 1)
                c_proj(st, nb)

            def phase_A(st, interleave=None, preloaded=False,
                        between_softmax=None):
                # interleave: optional callable(tb) emitting prev-img C blocks
                img = st["img"]

                for s in range(8):
                    h = s - 1
                    # x prefetch runs 2+ row-blocks ahead; when this image's
                    # first blocks were preloaded during the previous image's
                    # phase, only rb3..rb7 remain.
                    if preloaded:
                        if s <= 4:
                            load_rb(img, s + 3)
                    else:
                        if s == 0:
                            load_rb(img, 0)
                            load_rb(img, 1)
                            load_rb(img, 2)
                        elif s <= 5:
                            load_rb(img, s + 2)
                    # the NEXT image's first row-blocks load near the end of
                    # this phase (their xt regions are long since consumed).
                    if s >= 5 and img + 1 < IMGS:
                        load_rb(img + 1, s - 5, preload=True)
                    # PE-queue order spreads the two qk fronts apart (dwconv
                    # halves + v block between them) so each front's qp PSUM
                    # bank has a few microseconds of covering PE work before
                    # the rotation reuses it; each front's vector chain
                    # (square -> fold -> reduce -> rsqrt -> qskb) then never
                    # stalls the PE. The previous image's C block lands
                    # before the grams for the same reason.
                    dwconv_block(st, s, 0)
                    if h >= 0:
                        qk_front(st, 2 * h)
                    if h >= 2:
                        v_block(st, h - 2)
                    dwconv_block(st, s, 1)
                    if h >= 0:
                        qk_front(st, 2 * h + 1)
                    if interleave is not None:
                        interleave(s)
                    if h >= 0:
                        qk_gram(st, 2 * h)
                        qk_gram(st, 2 * h + 1)
                # drain: last qk body (h=7) + remaining v blocks
                qk_front(st, 14)
                v_block(st, 5)
                qk_front(st, 15)
                qk_gram(st, 14)
                qk_gram(st, 15)
                v_block(st, 6)
                v_block(st, 7)
                # both head-group grams finished accumulating in the shared
                # bank; softmax g0 now, g1 via the hook (the last image's
                # trailing loop inserts its g0 attn@v matmuls in between so
                # the PE covers g1's softmax chain).
                softmax_g(st, 0)
                if between_softmax is not None:
                    between_softmax()
                softmax_g(st, 1)

            import contextlib
            rep_engines = (mybir.EngineType.PE, mybir.EngineType.DVE,
                           mybir.EngineType.Activation, mybir.EngineType.SP,
                           mybir.EngineType.Pool)
            rep_ctx = (tc.For_i(0, reps, 1, hint_engines=rep_engines)
                       if reps > 1 else contextlib.nullcontext())
            with rep_ctx:
                prev = None
                for img in range(IMGS):
                    st = make_img_state(img)
                    if prev is None:
                        phase_A(st)
                    else:
                        pv = prev

                        def emit_c(tb, pv=pv):
                            if tb < 8:
                                c_block(pv, tb)
                        phase_A(st, interleave=emit_c, preloaded=True)
                    prev = st
                for nb in range(8):
                    c_av(prev, nb, 0, tail=True)
                    c_av(prev, nb, 1, tail=True)
                    c_proj(prev, nb)

    nc.finalize()
    return nc


def _prep_consts(dw_kernel, bn_gamma, bn_beta, pw_kernel, q_bias, v_bias,
                 scale, proj_w, proj_b):
    taps_w = np.empty((9, C), np.float32)
    for ti, (dh, dw) in enumerate(TAPS):
        taps_w[ti] = dw_kernel[dh + 1, dw + 1, 0, :] * bn_gamma

    diag = np.zeros((128, NCH, 9, 128), np.float16)
    idx = np.arange(128)
    for cch in range(NCH):
        for ti in range(9):
            diag[idx, cch, ti, idx] = taps_w[ti, cch * 128 + idx].astype(np.float16)

    pwqk = np.empty((128, NCH, 512), np.float16)
    pwv = np.empty((128, NCH, NCH, 128), np.float16)
    for kc in range(NCH):
        pwqk[:, kc, :] = pw_kernel[kc * 128:(kc + 1) * 128, 0:512].astype(np.float16)
        for vc in range(NCH):
            pwv[:, kc, vc, :] = pw_kernel[kc * 128:(kc + 1) * 128,
                                          512 + vc * 128:512 + (vc + 1) * 128].astype(np.float16)

    projw = np.empty((128, NCH, 256), np.float16)
    for kc in range(NCH):
        projw[:, kc, :] = proj_w[kc * 128:(kc + 1) * 128, :].astype(np.float16)

    s_host = np.exp(np.minimum(scale.reshape(HEADS), LOG_MAX_SCALE)).astype(np.float32)

    # tap weights as per-partition columns [128, NCH, 9] for vector-engine
    # dwconv tap offload (scalar_tensor_tensor / activation scale operands)
    kcol = np.empty((128, NCH, 9), np.float32)
    for cch in range(NCH):
        for ti in range(9):
            kcol[:, cch, ti] = taps_w[ti, cch * 128:(cch + 1) * 128]

    consts = {
        "diag": diag,
        "pwqk": pwqk,
        "pwv": pwv,
        "projw": projw,
        "beta": bn_beta.reshape(NCH, 128).T.astype(np.float32).copy(),
        "vb": v_bias.reshape(NCH, 128).T.astype(np.float32).copy(),
        "kcol": kcol,
        "s_host": s_host,
        "qb": np.tile(q_bias[None, :], (128, 1)).astype(np.float32),
        "pb": np.tile(proj_b[None, :], (128, 1)).astype(np.float32),
    }
    return consts


def make_in_maps(x):
    # per-core channel-major fp16 x: [IMGS, C, N]
    xs = np.ascontiguousarray(
        np.asarray(x, np.float32).reshape(NCORES, IMGS, N, C).transpose(
            0, 1, 3, 2).astype(np.float16))
    return [{"x": xs[i]} for i in range(NCORES)]


def kernel(x, dw_kernel, bn_gamma, bn_beta, pw_kernel, q_bias, v_bias, scale,
           proj_w, proj_b):
    consts = _prep_consts(
        np.asarray(dw_kernel, np.float32), np.asarray(bn_gamma, np.float32),
        np.asarray(bn_beta, np.float32), np.asarray(pw_kernel, np.float32),
        np.asarray(q_bias, np.float32), np.asarray(v_bias, np.float32),
        np.asarray(scale, np.float32), np.asarray(proj_w, np.float32),
        np.asarray(proj_b, np.float32))

    add_qbias = bool(np.any(q_bias))
    add_pbias = bool(np.any(proj_b))
    nc = _build_program(consts, add_qbias, add_pbias)

    in_maps = make_in_maps(x)
    res = run_bass_kernel_spmd(nc, in_maps, core_ids=list(range(NCORES)))
    out = np.stack([res.results[i]["out"] for i in range(NCORES)])
    return out.reshape(B, H, W, C)


if __name__ == "__main__":
    pass



# revision 26
# speedup vs baseline: 1.0065x; 1.0065x over previous
"""Trainium2 Bass kernel for CHMSA (cross-covariance multi-head self-attention
with a ConvNorm qkv stem).

Problem (hardcoded):
  x         [16, 64, 64, 256] f32
  dw_kernel [3, 3, 1, 256]    depthwise 3x3, SAME
  bn_gamma/bn_beta [256]      per-channel affine after dwconv
  pw_kernel [256, 768]        1x1 conv -> qkv
  q_bias/v_bias [256]         qkv bias = concat([q_bias, 0, v_bias])
  scale     [8,1,1]           per-head logit scale, s = exp(min(scale, ln 100))
  proj_w    [256, 256], proj_b [256]

Sharding: pure data-parallel over batch: 16 images / 8 cores = 2 images/core.
No collectives.

Everything upstream of PSUM runs in fp16 (e5m10): fp16's 10-bit mantissa
keeps the end-to-end rel err at ~2.8e-3 (bf16's 7-bit mantissa blows the
2e-2 budget through the weight tensors), while every stationary operand
gets the 2-byte half-cost LDWEIGHTS and x DMA traffic halves. All PSUM
accumulation stays f32.

Per-core dataflow (per image, N = 4096 tokens, C = 256):
  1. x is pre-transposed to channel-major [C, N] and cast to fp16 on the
     HOST (make_in_maps); the xt tile is UNPADDED so each x row-block DMA
     is one contiguous 1KB-per-partition run (a zero-halo layout produced
     128-byte bursts that starved the dwconv at startup).
  2. dwconv: 9 diagonal fp16 matmuls per PSUM tile (channel-major), gamma
     folded into the diagonal weights. SAME padding via per-tap AP
     clipping: edge psum rows/cols simply don't receive out-of-image tap
     contributions. The two channel chunks are emitted as separate halves
     so qk fronts can interleave between them on the PE queue.
  3. qkv: q,k token-major per token-chunk PAIR: ACT square -> GpSimd
     half-fold -> DVE grouped reduce -> rsqrt per half (1/|q|, 1/|k|) ->
     ONE DVE op per chunk scales the whole 512-wide psum tile into the
     fp16 qskb tile (q-half by 1/|q|, k-half by 1/|k|; the per-head scale
     s is folded into the softmax logit gather instead); v channel-major.
  4. attn gram: fp16 [128,128] matmuls accumulated over all 32 token
     chunks. BOTH head-groups accumulate concurrently in one PSUM bank
     (att is [128, 256] = 1KB of the 2KB bank; start_tensor_calc zeroes
     the whole bank, so only the very first matmul starts and only the
     very last stops). Softmax applies s_h during the DVE logit gather;
     32x32 DVE transposes build attn^T (fp16).
  5. out_cm = attn^T-weighted v (channel-major, fp16), proj back to
     token-major (fp16 weights), DMA out. The previous image's C blocks
     interleave into the next image's phase between that stage's qk
     front and gram so the PE never stalls on the previous softmax, and
     the next image's first x row-blocks prefetch during this image's
     late stages (region-level dependency tracking permits it).
"""

import math

import numpy as np
import ml_dtypes

import concourse.bass as bass
import concourse.mybir as mybir
import concourse.tile as tile
from concourse import bacc
from concourse.bass_utils import run_bass_kernel_spmd

F32 = mybir.dt.float32
F32R = mybir.dt.float32r
F16 = mybir.dt.float16
BF16 = mybir.dt.bfloat16
AF = mybir.ActivationFunctionType
ALU = mybir.AluOpType

B, H, W, C = 16, 64, 64, 256
N = H * W              # 4096 tokens per image
HEADS = 8
HD = C // HEADS        # 32
NCORES = 8
IMGS = B // NCORES     # 2 images per core
NCH = C // 128         # 2 channel chunks
LOG_MAX_SCALE = float(np.log(100.0))

# dwconv tap offsets (dh, dw), center first so it can carry start=True with
# full-tile coverage; the ragged edge taps then accumulate.
TAPS = [(0, 0), (-1, -1), (-1, 0), (-1, 1), (0, -1), (0, 1), (1, -1), (1, 0), (1, 1)]

HBLK = 8               # h-rows per dwconv psum tile -> free dim 8*64 = 512
NBLK = N // 128        # 32 token chunks of 128

# ---- engine assignment knobs ----
# (GpSimd cannot read PSUM, so all PSUM evictions live on ACT/DVE.)
VT_EVICT_DVE = False    # DVE queue feeds the qk chain; keep v eviction on ACT


def _build_program(consts, add_qbias, add_pbias, reps=1):
    nc = bacc.Bacc()

    # x arrives channel-major ([IMGS, C, N], transposed + cast to fp16 on
    # the HOST) so the dwconv input tile loads directly with no PE
    # transposes; fp16 halves the x DMA traffic and makes every stationary
    # weight load a 2-byte (half-cost) LDWEIGHTS.
    x_dr = nc.dram_tensor("x", [IMGS, C, N], F16, kind="ExternalInput")
    out_dr = nc.dram_tensor("out", [IMGS, N, C], F32, kind="ExternalOutput")

    diag_dr = nc.inline_tensor(consts["diag"], "cdiag")        # [128, NCH, 9, 128]
    pwqk_dr = nc.inline_tensor(consts["pwqk"], "cpwqk")        # [128, NCH, 512]
    pwv_dr = nc.inline_tensor(consts["pwv"], "cpwv")           # [128, NCH, NCH, 128]
    projw_dr = nc.inline_tensor(consts["projw"], "cprojw")     # [128, NCH, 256] bf16
    beta_dr = nc.inline_tensor(consts["beta"], "cbeta")        # [128, NCH]
    vb_dr = nc.inline_tensor(consts["vb"], "cvb")              # [128, NCH]
    s_host = [float(v) for v in consts["s_host"]]              # python floats
    if add_qbias:
        qb_dr = nc.inline_tensor(consts["qb"], "cqb")          # [128, 256]
    if add_pbias:
        pb_dr = nc.inline_tensor(consts["pb"], "cpb")          # [128, 256]

    use_beta = bool(np.any(consts["beta"]))
    use_vb = bool(np.any(consts["vb"]))

    with tile.TileContext(nc) as tc:
        with (
            tc.tile_pool(name="singles", bufs=1) as singles,
            tc.tile_pool(name="xt", bufs=1) as xt_pool,
            tc.tile_pool(name="img_big", bufs=1) as img_pool,
            tc.tile_pool(name="sq", bufs=4) as sq_pool,
            tc.tile_pool(name="wp", bufs=4) as wp_pool,
            tc.tile_pool(name="small", bufs=3) as small,
            tc.tile_pool(name="ostage", bufs=6) as ostage,
            tc.tile_pool(name="ps_mm", bufs=2, space="PSUM") as ps_mm,
            tc.tile_pool(name="ps_qk", bufs=3, space="PSUM") as ps_qk,
            tc.tile_pool(name="ps_c", bufs=2, space="PSUM") as ps_c,
            tc.tile_pool(name="ps_attn", bufs=1, space="PSUM") as ps_attn,
        ):
            # ---- constants into SBUF ----
            # Spread the big const DMAs across engine DGE queues so the
            # first dwconv tile isn't gated on one serial queue: diag's
            # two channel chunks go to the vector and scalar queues (the
            # sync + gpsimd queues carry the first x tiles).
            diag_sb = singles.tile([128, NCH, 9, 128], F16)
            nc.scalar.dma_start(diag_sb[:, 0], diag_dr[:, 0])
            nc.scalar.dma_start(diag_sb[:, 1], diag_dr[:, 1])
            pwqk_sb = singles.tile([128, NCH, 512], F16)
            nc.scalar.dma_start(pwqk_sb[:], pwqk_dr[:])
            pwv_sb = singles.tile([128, NCH, NCH, 128], F16)
            nc.scalar.dma_start(pwv_sb[:], pwv_dr[:])
            projw_sb = singles.tile([128, NCH, 256], F16)
            nc.scalar.dma_start(projw_sb[:], projw_dr[:])
            beta_sb = singles.tile([128, NCH], F32)
            nc.scalar.dma_start(beta_sb[:], beta_dr[:])
            vb_sb = singles.tile([128, NCH], F32)
            nc.scalar.dma_start(vb_sb[:], vb_dr[:])
            if add_qbias:
                qb_sb = singles.tile([128, 256], F32)
                nc.gpsimd.dma_start(qb_sb[:], qb_dr[:])
            if add_pbias:
                pb_sb = singles.tile([128, 256], F32)
                nc.gpsimd.dma_start(pb_sb[:], pb_dr[:])

            # xt is shared by both images (re-DMA'd per image), UNPADDED:
            # SAME-padding is expressed by clipping each tap's matmul APs
            # instead of a zero halo, so every x DMA is a contiguous
            # 1KB-per-partition run (the padded layout produced 128-byte
            # bursts that starved the dwconv at startup).
            xt_sh = xt_pool.tile([128, NCH, H, W], F16,
                                 tag="xt", name="xt_sh")

            def make_img_state(img):
                st = {}
                st["img"] = img
                st["xt"] = xt_sh
                st["yt"] = img_pool.tile([128, NCH, N], F16, tag="yt",
                                         name=f"yt{img}")
                st["vt"] = img_pool.tile([128, NCH, N], F16, tag="vt",
                                         name=f"vt{img}")
                # one PSUM bank holds BOTH head-groups' grams ([128, 256]
                # = 1KB of the 2KB bank). start_tensor_calc zeroes the
                # whole bank, so only the first matmul into the bank may
                # carry start=True and only the very last stop=True.
                st["att"] = ps_attn.tile([128, 2, 128], F32, tag="att",
                                         name=f"att_{img}")
                # [q-half | k-half] per token chunk, already l2-scaled
                st["qskb"] = img_pool.tile([128, NBLK, 512], F16, tag="qskb",
                                           name=f"qskb{img}")
                return st

            def load_rb(img, rb, preload=False):
                # DMA one 8-row block (512 tokens) of channel-major x into
                # the xt tile; the two channel chunks ride separate DGE
                # queues (sync + gpsimd) so they transfer in parallel.
                # Preloads for the NEXT image ride the scalar queue instead:
                # it is idle after the startup constants, while the sync
                # queue carries the previous image's out stores at exactly
                # that point in the schedule.
                for cch in range(NCH):
                    eng = nc.sync if cch == 0 else nc.gpsimd
                    eng.dma_start(
                        xt_sh[:, cch, 8 * rb:8 * (rb + 1), :].rearrange(
                            "p h w -> p (h w)"),
                        x_dr[img, cch * 128:(cch + 1) * 128,
                             rb * 512:(rb + 1) * 512],
                    )

            def dwconv_block(st, hb, cch):
                # SAME padding via AP clipping: each tap's matmul writes only
                # the psum rows/cols whose shifted input lies inside the
                # image; edge cells simply receive fewer tap contributions
                # (they were zeroed by the center tap's start_tensor_calc).
                # One channel chunk per call so the caller can interleave
                # other PE work between the halves.
                h0 = hb * HBLK
                if True:
                    ysl = st["yt"][:, cch, h0 * W:(h0 + HBLK) * W]
                    yp = ps_mm.tile([128, HBLK, W], F32, tag="mm", name="yp")
                    for i, ti in enumerate(range(9)):
                        dh, dw = TAPS[ti]
                        r0 = max(0, -(h0 + dh))
                        r1 = HBLK + min(0, H - (h0 + HBLK + dh))
                        c0 = max(0, -dw)
                        c1 = W - max(0, dw)
                        nc.tensor.matmul(
                            yp[:, r0:r1, c0:c1],
                            diag_sb[:, cch, ti, :],
                            st["xt"][:, cch, h0 + r0 + dh:h0 + r1 + dh,
                                     c0 + dw:c1 + dw],
                            start=(i == 0),
                            stop=(i == 8),
                            skip_group_check=True,
                        )
                    ypf = yp.rearrange("p h w -> p (h w)")
                    if use_beta:
                        nc.scalar.activation(
                            out=ysl, in_=ypf, func=AF.Identity,
                            bias=beta_sb[:, cch:cch + 1],
                        )
                    else:
                        nc.scalar.copy(ysl, ypf)

            def v_block(st, nb):
                for vc in range(NCH):
                    vp = ps_mm.tile([128, 512], F32, tag="mm", name="vp")
                    for kc in range(NCH):
                        nc.tensor.matmul(
                            vp[:],
                            pwv_sb[:, kc, vc, :],
                            st["yt"][:, kc, nb * 512:(nb + 1) * 512],
                            start=(kc == 0),
                            stop=(kc == NCH - 1),
                        )
                    vsl = st["vt"][:, vc, nb * 512:(nb + 1) * 512]
                    if VT_EVICT_DVE:
                        nc.vector.tensor_scalar(
                            out=vsl, in0=vp[:], scalar1=vb_sb[:, vc:vc + 1],
                            scalar2=None, op0=ALU.add,
                        )
                    elif use_vb:
                        nc.scalar.activation(
                            out=vsl, in_=vp[:], func=AF.Identity,
                            bias=vb_sb[:, vc:vc + 1],
                        )
                    else:
                        nc.scalar.copy(vsl, vp[:])

            def qk_front(st, p):
                # two token chunks t0,t1: qkv matmuls -> squares (ACT) ->
                # grouped reduce (GpSimd) -> per-half rsqrt (1/|q|, 1/|k|)
                # -> ONE DVE op per chunk writes the whole bf16 qskb tile
                # straight from PSUM (q-half scaled by 1/|q|, k-half by
                # 1/|k|). The gram matmuls are emitted later (qk_gram) so
                # other PE work covers this vector-side latency.
                qps = []
                sqs = []
                for j in (0, 1):
                    t = 2 * p + j
                    qp = ps_qk.tile([128, 512], F32, tag="qk", name=f"qp{j}")
                    for kc in range(NCH):
                        nc.tensor.matmul(
                            qp[:],
                            st["yt"][:, kc, t * 128:(t + 1) * 128],
                            pwqk_sb[:, kc, :],
                            start=(kc == 0),
                            stop=(kc == NCH - 1),
                        )
                    if add_qbias:
                        nc.vector.tensor_tensor(
                            out=qp[:, 0:256], in0=qp[:, 0:256],
                            in1=qb_sb[:], op=ALU.add,
                        )
                    sq = sq_pool.tile([128, 512], F32, name="sq")
                    nc.scalar.square(sq[:], qp[:])
                    qps.append(qp)
                    sqs.append(sq)
                sqr = wp_pool.tile([128, 2, 16], F32, tag="sqr", name="sqr")
                for j in (0, 1):
                    # single full-width DVE grouped reduce: one less engine
                    # hop in the front chain than the GpSimd-fold variant,
                    # which matters because the chain latency gates the qp
                    # PSUM bank rotation.
                    nc.vector.tensor_reduce(
                        out=sqr[:, j, :],
                        in_=sqs[j].rearrange("p (g d) -> p g d", d=HD),
                        axis=mybir.AxisListType.X,
                        op=ALU.add,
                    )
                # sqr = [|q|^2 (8 heads) | |k|^2 (8 heads)] per chunk;
                # in-place rsqrt gives exactly the per-half scale vector.
                sqf = sqr.rearrange("p a h -> p (a h)")
                nc.vector.reciprocal(sqf, sqf)
                nc.scalar.activation(sqf, sqf, AF.Sqrt)
                for j in (0, 1):
                    t = 2 * p + j
                    nc.vector.tensor_tensor(
                        out=st["qskb"][:, t, :].rearrange(
                            "p (g d) -> p g d", d=HD),
                        in0=qps[j].rearrange("p (g d) -> p g d", d=HD),
                        in1=sqr[:, j, :].unsqueeze(2).broadcast_to(
                            [128, 16, HD]),
                        op=ALU.mult,
                    )

            def qk_gram(st, p):
                # both head-groups accumulate in the shared att bank;
                # start only zeroes once (whole-bank zero), stop only on
                # the very last matmul into the bank.
                for j in (0, 1):
                    t = 2 * p + j
                    for g in range(2):
                        nc.tensor.matmul(
                            st["att"][:, g, :],
                            st["qskb"][:, t, g * 128:(g + 1) * 128],
                            st["qskb"][:, t, 256 + g * 128:256 + (g + 1) * 128],
                            start=(t == 0 and g == 0),
                            stop=(t == NBLK - 1 and g == 1),
                            skip_group_check=True,
                        )

            def softmax_g(st, g):
                if g == 0:
                    st["at_bd"] = small.tile([128, 2, 128], F16, tag="atbd",
                                             name="at_bd")
                at_bd = st["at_bd"]
                asm = small.tile([128, 32], F32, tag="asm", name="asm")
                for j in range(4):
                    h = 4 * g + j
                    nc.vector.tensor_scalar(
                        out=asm[32 * j:32 * j + 32, :],
                        in0=st["att"][32 * j:32 * j + 32, g, 32 * j:32 * j + 32],
                        scalar1=s_host[h], scalar2=None, op0=ALU.mult,
                    )
                mx = small.tile([128, 1], F32, tag="mx", name="mx")
                nc.vector.tensor_reduce(
                    out=mx[:], in_=asm[:], axis=mybir.AxisListType.X,
                    op=ALU.max, negate=True)
                nc.scalar.activation(asm[:], asm[:], AF.Exp, bias=mx[:])
                sm = small.tile([128, 1], F32, tag="sm", name="sm")
                nc.vector.tensor_reduce(
                    out=sm[:], in_=asm[:], axis=mybir.AxisListType.X,
                    op=ALU.add)
                nc.vector.reciprocal(sm[:], sm[:])
                nc.vector.tensor_scalar(
                    out=asm[:], in0=asm[:], scalar1=sm[:], scalar2=None,
                    op0=ALU.mult)
                atf = small.tile([128, 128], F32, tag="atf", name="atf")
                nc.vector.memset(atf[:], 0.0)
                for j in range(4):
                    nc.vector.transpose(
                        atf[32 * j:32 * j + 32, 32 * j:32 * j + 32],
                        asm[32 * j:32 * j + 32, :],
                    )
                nc.vector.tensor_copy(at_bd[:, g, :], atf[:])

            def get_ocm(st):
                img = st["img"]
                if "ocm" not in st:
                    st["ocm"] = img_pool.tile([128, NCH, N], F16, tag="ocm",
                                              name=f"ocm{img}")
                return st["ocm"]

            def c_av(st, nb, g, tail=False):
                # attn^T @ v for one head-group over one 512-token slab; in
                # the trailing loop the tiles borrow the idle ps_qk banks.
                ocm = get_ocm(st)
                op_ = (ps_qk if tail else ps_c).tile(
                    [128, 512], F32, tag="qk" if tail else "cmm",
                    name="op_")
                nc.tensor.matmul(
                    op_[:],
                    st["at_bd"][:, g, :],
                    st["vt"][:, g, nb * 512:(nb + 1) * 512],
                )
                if g == 0:
                    nc.vector.tensor_copy(
                        ocm[:, g, nb * 512:(nb + 1) * 512], op_[:])
                else:
                    nc.scalar.copy(
                        ocm[:, g, nb * 512:(nb + 1) * 512], op_[:])

            def c_proj(st, nb):
                img = st["img"]
                ocm = get_ocm(st)
                for t in range(4 * nb, 4 * nb + 4):
                    pp = ps_c.tile([128, 256], F32, tag="cmm", name="pp")
                    for kc in range(NCH):
                        nc.tensor.matmul(
                            pp[:],
                            ocm[:, kc, t * 128:(t + 1) * 128],
                            projw_sb[:, kc, :],
                            start=(kc == 0),
                            stop=(kc == NCH - 1),
                        )
                    if t % 2 == 0:
                        ot = ostage.tile([128, 2, 256], F32, name="ot")
                        st["ot"] = ot
                    ot = st["ot"]
                    if add_pbias:
                        nc.vector.tensor_tensor(
                            out=ot[:, t % 2, :], in0=pp[:], in1=pb_sb[:],
                            op=ALU.add)
                    elif t % 2 == 0:
                        nc.scalar.copy(ot[:, 0, :], pp[:])
                    else:
                        nc.vector.tensor_copy(ot[:, 1, :], pp[:])
                    if t % 2 == 1:
                        nc.sync.dma_start(
                            out_dr[img, (t - 1) * 128:(t + 1) * 128,
                                   :].rearrange("(g p) c -> p g c", p=128),
                            ot[:],
                        )

            def c_block(st, nb):
                c_av(st, nb, 0)
                c_av(st, nb, 1)
                c_proj(st, nb)

            def phase_A(st, interleave=None, preloaded=False,
                        between_softmax=None):
                # interleave: optional callable(tb) emitting prev-img C blocks
                img = st["img"]

                for s in range(8):
                    h = s - 1
                    # x prefetch runs 2+ row-blocks ahead; when this image's
                    # first blocks were preloaded during the previous image's
                    # phase, only rb3..rb7 remain.
                    if preloaded:
                        if s <= 4:
                            load_rb(img, s + 3)
                    else:
                        if s == 0:
                            load_rb(img, 0)
                            load_rb(img, 1)
                            load_rb(img, 2)
                        elif s <= 5:
                            load_rb(img, s + 2)
                    # the NEXT image's first row-blocks load near the end of
                    # this phase (their xt regions are long since consumed).
                    if s >= 5 and img + 1 < IMGS:
                        load_rb(img + 1, s - 5, preload=True)
                    # PE-queue order spreads the two qk fronts apart (dwconv
                    # halves + v block between them) so each front's qp PSUM
                    # bank has a few microseconds of covering PE work before
                    # the rotation reuses it; each front's vector chain
                    # (square -> fold -> reduce -> rsqrt -> qskb) then never
                    # stalls the PE. The previous image's C block lands
                    # before the grams for the same reason.
                    dwconv_block(st, s, 0)
                    if h >= 0:
                        qk_front(st, 2 * h)
                    if h >= 2:
                        v_block(st, h - 2)
                    dwconv_block(st, s, 1)
                    if h >= 0:
                        qk_front(st, 2 * h + 1)
                    if interleave is not None:
                        interleave(s)
                    if h >= 0:
                        qk_gram(st, 2 * h)
                        qk_gram(st, 2 * h + 1)
                # drain: last qk body (h=7) + remaining v blocks
                qk_front(st, 14)
                v_block(st, 5)
                qk_front(st, 15)
                qk_gram(st, 14)
                qk_gram(st, 15)
                v_block(st, 6)
                v_block(st, 7)
                # both head-group grams finished accumulating in the shared
                # bank; softmax g0 now, g1 via the hook (the last image's
                # trailing loop inserts its g0 attn@v matmuls in between so
                # the PE covers g1's softmax chain).
                softmax_g(st, 0)
                if between_softmax is not None:
                    between_softmax()
                softmax_g(st, 1)

            import contextlib
            rep_engines = (mybir.EngineType.PE, mybir.EngineType.DVE,
                           mybir.EngineType.Activation, mybir.EngineType.SP,
                           mybir.EngineType.Pool)
            rep_ctx = (tc.For_i(0, reps, 1, hint_engines=rep_engines)
                       if reps > 1 else contextlib.nullcontext())
            with rep_ctx:
                prev = None
                for img in range(IMGS):
                    st = make_img_state(img)
                    if prev is None:
                        phase_A(st)
                    else:
                        pv = prev

                        def emit_c(tb, pv=pv):
                            if tb < 8:
                                c_block(pv, tb)
                        phase_A(st, interleave=emit_c, preloaded=True)
                    prev = st
                for nb in range(8):
                    c_av(prev, nb, 0, tail=True)
                    c_av(prev, nb, 1, tail=True)
                    c_proj(prev, nb)

    nc.finalize()
    return nc


def _prep_consts(dw_kernel, bn_gamma, bn_beta, pw_kernel, q_bias, v_bias,
                 scale, proj_w, proj_b):
    taps_w = np.empty((9, C), np.float32)
    for ti, (dh, dw) in enumerate(TAPS):
        taps_w[ti] = dw_kernel[dh + 1, dw + 1, 0, :] * bn_gamma

    diag = np.zeros((128, NCH, 9, 128), np.float16)
    idx = np.arange(128)
    for cch in range(NCH):
        for ti in range(9):
            diag[idx, cch, ti, idx] = taps_w[ti, cch * 128 + idx].astype(np.float16)

    pwqk = np.empty((128, NCH, 512), np.float16)
    pwv = np.empty((128, NCH, NCH, 128), np.float16)
    for kc in range(NCH):
        pwqk[:, kc, :] = pw_kernel[kc * 128:(kc + 1) * 128, 0:512].astype(np.float16)
        for vc in range(NCH):
            pwv[:, kc, vc, :] = pw_kernel[kc * 128:(kc + 1) * 128,
                                          512 + vc * 128:512 + (vc + 1) * 128].astype(np.float16)

    projw = np.empty((128, NCH, 256), np.float16)
    for kc in range(NCH):
        projw[:, kc, :] = proj_w[kc * 128:(kc + 1) * 128, :].astype(np.float16)

    s_host = np.exp(np.minimum(scale.reshape(HEADS), LOG_MAX_SCALE)).astype(np.float32)

    # tap weights as per-partition columns [128, NCH, 9] for vector-engine
    # dwconv tap offload (scalar_tensor_tensor / activation scale operands)
    kcol = np.empty((128, NCH, 9), np.float32)
    for cch in range(NCH):
        for ti in range(9):
            kcol[:, cch, ti] = taps_w[ti, cch * 128:(cch + 1) * 128]

    consts = {
        "diag": diag,
        "pwqk": pwqk,
        "pwv": pwv,
        "projw": projw,
        "beta": bn_beta.reshape(NCH, 128).T.astype(np.float32).copy(),
        "vb": v_bias.reshape(NCH, 128).T.astype(np.float32).copy(),
        "kcol": kcol,
        "s_host": s_host,
        "qb": np.tile(q_bias[None, :], (128, 1)).astype(np.float32),
        "pb": np.tile(proj_b[None, :], (128, 1)).astype(np.float32),
    }
    return consts


def make_in_maps(x):
    # per-core channel-major fp16 x: [IMGS, C, N]
    xs = np.ascontiguousarray(
        np.asarray(x, np.float32).reshape(NCORES, IMGS, N, C).transpose(
            0, 1, 3, 2).astype(np.float16))
    return [{"x": xs[i]} for i in range(NCORES)]


def kernel(x, dw_kernel, bn_gamma, bn_beta, pw_kernel, q_bias, v_bias, scale,
           proj_w, proj_b):
    consts = _prep_consts(
        np.asarray(dw_kernel, np.float32), np.asarray(bn_gamma, np.float32),
        np.asarray(bn_beta, np.float32), np.asarray(pw_kernel, np.float32),
        np.asarray(q_bias, np.float32), np.asarray(v_bias, np.float32),
        np.asarray(scale, np.float32), np.asarray(proj_w, np.float32),
        np.asarray(proj_b, np.float32))

    add_qbias = bool(np.any(q_bias))
    add_pbias = bool(np.any(proj_b))
    nc = _build_program(consts, add_qbias, add_pbias)

    in_maps = make_in_maps(x)
    res = run_bass_kernel_spmd(nc, in_maps, core_ids=list(range(NCORES)))
    out = np.stack([res.results[i]["out"] for i in range(NCORES)])
    return out.reshape(B, H, W, C)


if __name__ == "__main__":
    pass



# revision 27
# speedup vs baseline: 1.0130x; 1.0064x over previous
"""Trainium2 Bass kernel for CHMSA (cross-covariance multi-head self-attention
with a ConvNorm qkv stem).

Problem (hardcoded):
  x         [16, 64, 64, 256] f32
  dw_kernel [3, 3, 1, 256]    depthwise 3x3, SAME
  bn_gamma/bn_beta [256]      per-channel affine after dwconv
  pw_kernel [256, 768]        1x1 conv -> qkv
  q_bias/v_bias [256]         qkv bias = concat([q_bias, 0, v_bias])
  scale     [8,1,1]           per-head logit scale, s = exp(min(scale, ln 100))
  proj_w    [256, 256], proj_b [256]

Sharding: pure data-parallel over batch: 16 images / 8 cores = 2 images/core.
No collectives.

Everything upstream of PSUM runs in fp16 (e5m10): fp16's 10-bit mantissa
keeps the end-to-end rel err at ~2.8e-3 (bf16's 7-bit mantissa blows the
2e-2 budget through the weight tensors), while every stationary operand
gets the 2-byte half-cost LDWEIGHTS and x DMA traffic halves. All PSUM
accumulation stays f32.

Per-core dataflow (per image, N = 4096 tokens, C = 256):
  1. x is pre-transposed to channel-major [C, N] and cast to fp16 on the
     HOST (make_in_maps); the xt tile is UNPADDED so each x row-block DMA
     is one contiguous 1KB-per-partition run (a zero-halo layout produced
     128-byte bursts that starved the dwconv at startup).
  2. dwconv: 9 diagonal fp16 matmuls per PSUM tile (channel-major), gamma
     folded into the diagonal weights. SAME padding via per-tap AP
     clipping: edge psum rows/cols simply don't receive out-of-image tap
     contributions. The two channel chunks are emitted as separate halves
     so qk fronts can interleave between them on the PE queue.
  3. qkv: q,k token-major per token-chunk PAIR: ACT square -> GpSimd
     half-fold -> DVE grouped reduce -> rsqrt per half (1/|q|, 1/|k|) ->
     ONE DVE op per chunk scales the whole 512-wide psum tile into the
     fp16 qskb tile (q-half by 1/|q|, k-half by 1/|k|; the per-head scale
     s is folded into the softmax logit gather instead); v channel-major.
  4. attn gram: fp16 [128,128] matmuls accumulated over all 32 token
     chunks. BOTH head-groups accumulate concurrently in one PSUM bank
     (att is [128, 256] = 1KB of the 2KB bank; start_tensor_calc zeroes
     the whole bank, so only the very first matmul starts and only the
     very last stops). Softmax applies s_h during the DVE logit gather;
     32x32 DVE transposes build attn^T (fp16).
  5. out_cm = attn^T-weighted v (channel-major, fp16), proj back to
     token-major (fp16 weights), DMA out. The previous image's C blocks
     interleave into the next image's phase between that stage's qk
     front and gram so the PE never stalls on the previous softmax, and
     the next image's first x row-blocks prefetch during this image's
     late stages (region-level dependency tracking permits it).
"""

import math

import numpy as np
import ml_dtypes

import concourse.bass as bass
import concourse.mybir as mybir
import concourse.tile as tile
from concourse import bacc
from concourse.bass_utils import run_bass_kernel_spmd

F32 = mybir.dt.float32
F32R = mybir.dt.float32r
F16 = mybir.dt.float16
BF16 = mybir.dt.bfloat16
AF = mybir.ActivationFunctionType
ALU = mybir.AluOpType

B, H, W, C = 16, 64, 64, 256
N = H * W              # 4096 tokens per image
HEADS = 8
HD = C // HEADS        # 32
NCORES = 8
IMGS = B // NCORES     # 2 images per core
NCH = C // 128         # 2 channel chunks
LOG_MAX_SCALE = float(np.log(100.0))

# dwconv tap offsets (dh, dw), center first so it can carry start=True with
# full-tile coverage; the ragged edge taps then accumulate.
TAPS = [(0, 0), (-1, -1), (-1, 0), (-1, 1), (0, -1), (0, 1), (1, -1), (1, 0), (1, 1)]

HBLK = 8               # h-rows per dwconv psum tile -> free dim 8*64 = 512
NBLK = N // 128        # 32 token chunks of 128

# ---- engine assignment knobs ----
# (GpSimd cannot read PSUM, so all PSUM evictions live on ACT/DVE.)
VT_EVICT_DVE = False    # DVE queue feeds the qk chain; keep v eviction on ACT


def _build_program(consts, add_qbias, add_pbias, reps=1):
    nc = bacc.Bacc()

    # x arrives channel-major ([IMGS, C, N], transposed + cast to fp16 on
    # the HOST) so the dwconv input tile loads directly with no PE
    # transposes; fp16 halves the x DMA traffic and makes every stationary
    # weight load a 2-byte (half-cost) LDWEIGHTS.
    x_dr = nc.dram_tensor("x", [IMGS, C, N], F16, kind="ExternalInput")
    out_dr = nc.dram_tensor("out", [IMGS, N, C], F32, kind="ExternalOutput")

    diag_dr = nc.inline_tensor(consts["diag"], "cdiag")        # [128, NCH, 9, 128]
    pwqk_dr = nc.inline_tensor(consts["pwqk"], "cpwqk")        # [128, NCH, 512]
    pwv_dr = nc.inline_tensor(consts["pwv"], "cpwv")           # [128, NCH, NCH, 128]
    projw_dr = nc.inline_tensor(consts["projw"], "cprojw")     # [128, NCH, 256] bf16
    beta_dr = nc.inline_tensor(consts["beta"], "cbeta")        # [128, NCH]
    vb_dr = nc.inline_tensor(consts["vb"], "cvb")              # [128, NCH]
    s_host = [float(v) for v in consts["s_host"]]              # python floats
    if add_qbias:
        qb_dr = nc.inline_tensor(consts["qb"], "cqb")          # [128, 256]
    if add_pbias:
        pb_dr = nc.inline_tensor(consts["pb"], "cpb")          # [128, 256]

    use_beta = bool(np.any(consts["beta"]))
    use_vb = bool(np.any(consts["vb"]))

    with tile.TileContext(nc) as tc:
        with (
            tc.tile_pool(name="singles", bufs=1) as singles,
            tc.tile_pool(name="xt", bufs=1) as xt_pool,
            tc.tile_pool(name="img_big", bufs=1) as img_pool,
            tc.tile_pool(name="sq", bufs=4) as sq_pool,
            tc.tile_pool(name="wp", bufs=4) as wp_pool,
            tc.tile_pool(name="small", bufs=3) as small,
            tc.tile_pool(name="ostage", bufs=6) as ostage,
            tc.tile_pool(name="ps_mm", bufs=2, space="PSUM") as ps_mm,
            tc.tile_pool(name="ps_qk", bufs=3, space="PSUM") as ps_qk,
            tc.tile_pool(name="ps_c", bufs=2, space="PSUM") as ps_c,
            tc.tile_pool(name="ps_attn", bufs=1, space="PSUM") as ps_attn,
        ):
            # ---- constants into SBUF ----
            # Spread the big const DMAs across engine DGE queues so the
            # first dwconv tile isn't gated on one serial queue: diag's
            # two channel chunks go to the vector and scalar queues (the
            # sync + gpsimd queues carry the first x tiles).
            diag_sb = singles.tile([128, NCH, 9, 128], F16)
            nc.scalar.dma_start(diag_sb[:, 0], diag_dr[:, 0])
            nc.scalar.dma_start(diag_sb[:, 1], diag_dr[:, 1])
            pwqk_sb = singles.tile([128, NCH, 512], F16)
            nc.scalar.dma_start(pwqk_sb[:], pwqk_dr[:])
            pwv_sb = singles.tile([128, NCH, NCH, 128], F16)
            nc.scalar.dma_start(pwv_sb[:], pwv_dr[:])
            projw_sb = singles.tile([128, NCH, 256], F16)
            nc.scalar.dma_start(projw_sb[:], projw_dr[:])
            beta_sb = singles.tile([128, NCH], F32)
            nc.scalar.dma_start(beta_sb[:], beta_dr[:])
            vb_sb = singles.tile([128, NCH], F32)
            nc.scalar.dma_start(vb_sb[:], vb_dr[:])
            if add_qbias:
                qb_sb = singles.tile([128, 256], F32)
                nc.gpsimd.dma_start(qb_sb[:], qb_dr[:])
            if add_pbias:
                pb_sb = singles.tile([128, 256], F32)
                nc.gpsimd.dma_start(pb_sb[:], pb_dr[:])

            # xt is shared by both images (re-DMA'd per image), UNPADDED:
            # SAME-padding is expressed by clipping each tap's matmul APs
            # instead of a zero halo, so every x DMA is a contiguous
            # 1KB-per-partition run (the padded layout produced 128-byte
            # bursts that starved the dwconv at startup).
            xt_sh = xt_pool.tile([128, NCH, H, W], F16,
                                 tag="xt", name="xt_sh")

            def make_img_state(img):
                st = {}
                st["img"] = img
                st["xt"] = xt_sh
                st["yt"] = img_pool.tile([128, NCH, N], F16, tag="yt",
                                         name=f"yt{img}")
                st["vt"] = img_pool.tile([128, NCH, N], F16, tag="vt",
                                         name=f"vt{img}")
                # one PSUM bank holds BOTH head-groups' grams ([128, 256]
                # = 1KB of the 2KB bank). start_tensor_calc zeroes the
                # whole bank, so only the first matmul into the bank may
                # carry start=True and only the very last stop=True.
                st["att"] = ps_attn.tile([128, 2, 128], F32, tag="att",
                                         name=f"att_{img}")
                # [q-half | k-half] per token chunk, already l2-scaled
                st["qskb"] = img_pool.tile([128, NBLK, 512], F16, tag="qskb",
                                           name=f"qskb{img}")
                return st

            def load_rb(img, rb, preload=False):
                # DMA one 8-row block (512 tokens) of channel-major x into
                # the xt tile; the two channel chunks ride separate DGE
                # queues (sync + gpsimd) so they transfer in parallel.
                # Preloads for the NEXT image ride the scalar queue instead:
                # it is idle after the startup constants, while the sync
                # queue carries the previous image's out stores at exactly
                # that point in the schedule.
                for cch in range(NCH):
                    eng = nc.sync if cch == 0 else nc.gpsimd
                    eng.dma_start(
                        xt_sh[:, cch, 8 * rb:8 * (rb + 1), :].rearrange(
                            "p h w -> p (h w)"),
                        x_dr[img, cch * 128:(cch + 1) * 128,
                             rb * 512:(rb + 1) * 512],
                    )

            def dwconv_block(st, hb, cch):
                # SAME padding via AP clipping: each tap's matmul writes only
                # the psum rows/cols whose shifted input lies inside the
                # image; edge cells simply receive fewer tap contributions
                # (they were zeroed by the center tap's start_tensor_calc).
                # One channel chunk per call so the caller can interleave
                # other PE work between the halves.
                h0 = hb * HBLK
                if True:
                    ysl = st["yt"][:, cch, h0 * W:(h0 + HBLK) * W]
                    yp = ps_mm.tile([128, HBLK, W], F32, tag="mm", name="yp")
                    for i, ti in enumerate(range(9)):
                        dh, dw = TAPS[ti]
                        r0 = max(0, -(h0 + dh))
                        r1 = HBLK + min(0, H - (h0 + HBLK + dh))
                        c0 = max(0, -dw)
                        c1 = W - max(0, dw)
                        nc.tensor.matmul(
                            yp[:, r0:r1, c0:c1],
                            diag_sb[:, cch, ti, :],
                            st["xt"][:, cch, h0 + r0 + dh:h0 + r1 + dh,
                                     c0 + dw:c1 + dw],
                            start=(i == 0),
                            stop=(i == 8),
                            skip_group_check=True,
                        )
                    ypf = yp.rearrange("p h w -> p (h w)")
                    if use_beta:
                        nc.scalar.activation(
                            out=ysl, in_=ypf, func=AF.Identity,
                            bias=beta_sb[:, cch:cch + 1],
                        )
                    else:
                        nc.scalar.copy(ysl, ypf)

            def v_block(st, nb):
                for vc in range(NCH):
                    vp = ps_mm.tile([128, 512], F32, tag="mm", name="vp")
                    for kc in range(NCH):
                        nc.tensor.matmul(
                            vp[:],
                            pwv_sb[:, kc, vc, :],
                            st["yt"][:, kc, nb * 512:(nb + 1) * 512],
                            start=(kc == 0),
                            stop=(kc == NCH - 1),
                        )
                    vsl = st["vt"][:, vc, nb * 512:(nb + 1) * 512]
                    if VT_EVICT_DVE:
                        nc.vector.tensor_scalar(
                            out=vsl, in0=vp[:], scalar1=vb_sb[:, vc:vc + 1],
                            scalar2=None, op0=ALU.add,
                        )
                    elif use_vb:
                        nc.scalar.activation(
                            out=vsl, in_=vp[:], func=AF.Identity,
                            bias=vb_sb[:, vc:vc + 1],
                        )
                    else:
                        nc.scalar.copy(vsl, vp[:])

            def qk_front(st, p):
                # two token chunks t0,t1: qkv matmuls -> squares (ACT) ->
                # grouped reduce (GpSimd) -> per-half rsqrt (1/|q|, 1/|k|)
                # -> ONE DVE op per chunk writes the whole bf16 qskb tile
                # straight from PSUM (q-half scaled by 1/|q|, k-half by
                # 1/|k|). The gram matmuls are emitted later (qk_gram) so
                # other PE work covers this vector-side latency.
                qps = []
                sqs = []
                for j in (0, 1):
                    t = 2 * p + j
                    qp = ps_qk.tile([128, 512], F32, tag="qk", name=f"qp{j}")
                    for kc in range(NCH):
                        nc.tensor.matmul(
                            qp[:],
                            st["yt"][:, kc, t * 128:(t + 1) * 128],
                            pwqk_sb[:, kc, :],
                            start=(kc == 0),
                            stop=(kc == NCH - 1),
                        )
                    if add_qbias:
                        nc.vector.tensor_tensor(
                            out=qp[:, 0:256], in0=qp[:, 0:256],
                            in1=qb_sb[:], op=ALU.add,
                        )
                    sq = sq_pool.tile([128, 512], F32, name="sq")
                    nc.scalar.square(sq[:], qp[:])
                    qps.append(qp)
                    sqs.append(sq)
                sqr = wp_pool.tile([128, 2, 16], F32, tag="sqr", name="sqr")
                for j in (0, 1):
                    # single full-width DVE grouped reduce: one less engine
                    # hop in the front chain than the GpSimd-fold variant,
                    # which matters because the chain latency gates the qp
                    # PSUM bank rotation.
                    nc.vector.tensor_reduce(
                        out=sqr[:, j, :],
                        in_=sqs[j].rearrange("p (g d) -> p g d", d=HD),
                        axis=mybir.AxisListType.X,
                        op=ALU.add,
                    )
                # sqr = [|q|^2 (8 heads) | |k|^2 (8 heads)] per chunk;
                # in-place rsqrt gives exactly the per-half scale vector.
                sqf = sqr.rearrange("p a h -> p (a h)")
                # sqrt on ACT first, reciprocal on DVE second: the final two
                # chain links (recip -> scaled eviction) then sit adjacent on
                # the DVE queue with no cross-engine hop before the op that
                # frees the qp PSUM bank.
                nc.scalar.activation(sqf, sqf, AF.Sqrt)
                nc.vector.reciprocal(sqf, sqf)
                for j in (0, 1):
                    t = 2 * p + j
                    nc.vector.tensor_tensor(
                        out=st["qskb"][:, t, :].rearrange(
                            "p (g d) -> p g d", d=HD),
                        in0=qps[j].rearrange("p (g d) -> p g d", d=HD),
                        in1=sqr[:, j, :].unsqueeze(2).broadcast_to(
                            [128, 16, HD]),
                        op=ALU.mult,
                    )

            def qk_gram(st, p):
                # both head-groups accumulate in the shared att bank;
                # start only zeroes once (whole-bank zero), stop only on
                # the very last matmul into the bank.
                for j in (0, 1):
                    t = 2 * p + j
                    for g in range(2):
                        nc.tensor.matmul(
                            st["att"][:, g, :],
                            st["qskb"][:, t, g * 128:(g + 1) * 128],
                            st["qskb"][:, t, 256 + g * 128:256 + (g + 1) * 128],
                            start=(t == 0 and g == 0),
                            stop=(t == NBLK - 1 and g == 1),
                            skip_group_check=True,
                        )

            def softmax_g(st, g):
                if g == 0:
                    st["at_bd"] = small.tile([128, 2, 128], F16, tag="atbd",
                                             name="at_bd")
                at_bd = st["at_bd"]
                asm = small.tile([128, 32], F32, tag="asm", name="asm")
                for j in range(4):
                    h = 4 * g + j
                    nc.vector.tensor_scalar(
                        out=asm[32 * j:32 * j + 32, :],
                        in0=st["att"][32 * j:32 * j + 32, g, 32 * j:32 * j + 32],
                        scalar1=s_host[h], scalar2=None, op0=ALU.mult,
                    )
                mx = small.tile([128, 1], F32, tag="mx", name="mx")
                nc.vector.tensor_reduce(
                    out=mx[:], in_=asm[:], axis=mybir.AxisListType.X,
                    op=ALU.max, negate=True)
                nc.scalar.activation(asm[:], asm[:], AF.Exp, bias=mx[:])
                sm = small.tile([128, 1], F32, tag="sm", name="sm")
                nc.vector.tensor_reduce(
                    out=sm[:], in_=asm[:], axis=mybir.AxisListType.X,
                    op=ALU.add)
                nc.vector.reciprocal(sm[:], sm[:])
                nc.vector.tensor_scalar(
                    out=asm[:], in0=asm[:], scalar1=sm[:], scalar2=None,
                    op0=ALU.mult)
                atf = small.tile([128, 128], F32, tag="atf", name="atf")
                nc.vector.memset(atf[:], 0.0)
                for j in range(4):
                    nc.vector.transpose(
                        atf[32 * j:32 * j + 32, 32 * j:32 * j + 32],
                        asm[32 * j:32 * j + 32, :],
                    )
                nc.vector.tensor_copy(at_bd[:, g, :], atf[:])

            def get_ocm(st):
                img = st["img"]
                if "ocm" not in st:
                    st["ocm"] = img_pool.tile([128, NCH, N], F16, tag="ocm",
                                              name=f"ocm{img}")
                return st["ocm"]

            def c_av(st, nb, g, tail=False):
                # attn^T @ v for one head-group over one 512-token slab; in
                # the trailing loop the tiles borrow the idle ps_qk banks.
                ocm = get_ocm(st)
                op_ = (ps_qk if tail else ps_c).tile(
                    [128, 512], F32, tag="qk" if tail else "cmm",
                    name="op_")
                nc.tensor.matmul(
                    op_[:],
                    st["at_bd"][:, g, :],
                    st["vt"][:, g, nb * 512:(nb + 1) * 512],
                )
                if g == 0:
                    nc.vector.tensor_copy(
                        ocm[:, g, nb * 512:(nb + 1) * 512], op_[:])
                else:
                    nc.scalar.copy(
                        ocm[:, g, nb * 512:(nb + 1) * 512], op_[:])

            def c_proj(st, nb):
                img = st["img"]
                ocm = get_ocm(st)
                for t in range(4 * nb, 4 * nb + 4):
                    pp = ps_c.tile([128, 256], F32, tag="cmm", name="pp")
                    for kc in range(NCH):
                        nc.tensor.matmul(
                            pp[:],
                            ocm[:, kc, t * 128:(t + 1) * 128],
                            projw_sb[:, kc, :],
                            start=(kc == 0),
                            stop=(kc == NCH - 1),
                        )
                    if t % 2 == 0:
                        ot = ostage.tile([128, 2, 256], F32, name="ot")
                        st["ot"] = ot
                    ot = st["ot"]
                    if add_pbias:
                        nc.vector.tensor_tensor(
                            out=ot[:, t % 2, :], in0=pp[:], in1=pb_sb[:],
                            op=ALU.add)
                    elif t % 2 == 0:
                        nc.scalar.copy(ot[:, 0, :], pp[:])
                    else:
                        nc.vector.tensor_copy(ot[:, 1, :], pp[:])
                    if t % 2 == 1:
                        nc.sync.dma_start(
                            out_dr[img, (t - 1) * 128:(t + 1) * 128,
                                   :].rearrange("(g p) c -> p g c", p=128),
                            ot[:],
                        )

            def c_block(st, nb):
                c_av(st, nb, 0)
                c_av(st, nb, 1)
                c_proj(st, nb)

            def phase_A(st, interleave=None, preloaded=False,
                        between_softmax=None):
                # interleave: optional callable(tb) emitting prev-img C blocks
                img = st["img"]

                for s in range(8):
                    h = s - 1
                    # x prefetch runs 2+ row-blocks ahead; when this image's
                    # first blocks were preloaded during the previous image's
                    # phase, only rb3..rb7 remain.
                    if preloaded:
                        if s <= 4:
                            load_rb(img, s + 3)
                    else:
                        if s == 0:
                            load_rb(img, 0)
                            load_rb(img, 1)
                            load_rb(img, 2)
                        elif s <= 5:
                            load_rb(img, s + 2)
                    # the NEXT image's first row-blocks load near the end of
                    # this phase (their xt regions are long since consumed).
                    if s >= 5 and img + 1 < IMGS:
                        load_rb(img + 1, s - 5, preload=True)
                    # PE-queue order spreads the two qk fronts apart (dwconv
                    # halves + v block between them) so each front's qp PSUM
                    # bank has a few microseconds of covering PE work before
                    # the rotation reuses it; each front's vector chain
                    # (square -> fold -> reduce -> rsqrt -> qskb) then never
                    # stalls the PE. The previous image's C block lands
                    # before the grams for the same reason.
                    dwconv_block(st, s, 0)
                    if h >= 0:
                        qk_front(st, 2 * h)
                    if h >= 2:
                        v_block(st, h - 2)
                    dwconv_block(st, s, 1)
                    if h >= 0:
                        qk_front(st, 2 * h + 1)
                    if interleave is not None:
                        interleave(s)
                    if h >= 0:
                        qk_gram(st, 2 * h)
                        qk_gram(st, 2 * h + 1)
                # drain: last qk body (h=7) + remaining v blocks
                qk_front(st, 14)
                v_block(st, 5)
                qk_front(st, 15)
                qk_gram(st, 14)
                qk_gram(st, 15)
                v_block(st, 6)
                v_block(st, 7)
                # both head-group grams finished accumulating in the shared
                # bank; softmax g0 now, g1 via the hook (the last image's
                # trailing loop inserts its g0 attn@v matmuls in between so
                # the PE covers g1's softmax chain).
                softmax_g(st, 0)
                if between_softmax is not None:
                    between_softmax()
                softmax_g(st, 1)

            import contextlib
            rep_engines = (mybir.EngineType.PE, mybir.EngineType.DVE,
                           mybir.EngineType.Activation, mybir.EngineType.SP,
                           mybir.EngineType.Pool)
            rep_ctx = (tc.For_i(0, reps, 1, hint_engines=rep_engines)
                       if reps > 1 else contextlib.nullcontext())
            with rep_ctx:
                prev = None
                for img in range(IMGS):
                    st = make_img_state(img)
                    if prev is None:
                        phase_A(st)
                    else:
                        pv = prev

                        def emit_c(tb, pv=pv):
                            if tb < 8:
                                c_block(pv, tb)
                        phase_A(st, interleave=emit_c, preloaded=True)
                    prev = st
                for nb in range(8):
                    c_av(prev, nb, 0, tail=True)
                    c_av(prev, nb, 1, tail=True)
                    c_proj(prev, nb)

    nc.finalize()
    return nc


def _prep_consts(dw_kernel, bn_gamma, bn_beta, pw_kernel, q_bias, v_bias,
                 scale, proj_w, proj_b):
    taps_w = np.empty((9, C), np.float32)
    for ti, (dh, dw) in enumerate(TAPS):
        taps_w[ti] = dw_kernel[dh + 1, dw + 1, 0, :] * bn_gamma

    diag = np.zeros((128, NCH, 9, 128), np.float16)
    idx = np.arange(128)
    for cch in range(NCH):
        for ti in range(9):
            diag[idx, cch, ti, idx] = taps_w[ti, cch * 128 + idx].astype(np.float16)

    pwqk = np.empty((128, NCH, 512), np.float16)
    pwv = np.empty((128, NCH, NCH, 128), np.float16)
    for kc in range(NCH):
        pwqk[:, kc, :] = pw_kernel[kc * 128:(kc + 1) * 128, 0:512].astype(np.float16)
        for vc in range(NCH):
            pwv[:, kc, vc, :] = pw_kernel[kc * 128:(kc + 1) * 128,
                                          512 + vc * 128:512 + (vc + 1) * 128].astype(np.float16)

    projw = np.empty((128, NCH, 256), np.float16)
    for kc in range(NCH):
        projw[:, kc, :] = proj_w[kc * 128:(kc + 1) * 128, :].astype(np.float16)

    s_host = np.exp(np.minimum(scale.reshape(HEADS), LOG_MAX_SCALE)).astype(np.float32)

    # tap weights as per-partition columns [128, NCH, 9] for vector-engine
    # dwconv tap offload (scalar_tensor_tensor / activation scale operands)
    kcol = np.empty((128, NCH, 9), np.float32)
    for cch in range(NCH):
        for ti in range(9):
            kcol[:, cch, ti] = taps_w[ti, cch * 128:(cch + 1) * 128]

    consts = {
        "diag": diag,
        "pwqk": pwqk,
        "pwv": pwv,
        "projw": projw,
        "beta": bn_beta.reshape(NCH, 128).T.astype(np.float32).copy(),
        "vb": v_bias.reshape(NCH, 128).T.astype(np.float32).copy(),
        "kcol": kcol,
        "s_host": s_host,
        "qb": np.tile(q_bias[None, :], (128, 1)).astype(np.float32),
        "pb": np.tile(proj_b[None, :], (128, 1)).astype(np.float32),
    }
    return consts


def make_in_maps(x):
    # per-core channel-major fp16 x: [IMGS, C, N]
    xs = np.ascontiguousarray(
        np.asarray(x, np.float32).reshape(NCORES, IMGS, N, C).transpose(
            0, 1, 3, 2).astype(np.float16))
    return [{"x": xs[i]} for i in range(NCORES)]


def kernel(x, dw_kernel, bn_gamma, bn_beta, pw_kernel, q_bias, v_bias, scale,
           proj_w, proj_b):
    consts = _prep_consts(
        np.asarray(dw_kernel, np.float32), np.asarray(bn_gamma, np.float32),
        np.asarray(bn_beta, np.float32), np.asarray(pw_kernel, np.float32),
        np.asarray(q_bias, np.float32), np.asarray(v_bias, np.float32),
        np.asarray(scale, np.float32), np.asarray(proj_w, np.float32),
        np.asarray(proj_b, np.float32))

    add_qbias = bool(np.any(q_bias))
    add_pbias = bool(np.any(proj_b))
    nc = _build_program(consts, add_qbias, add_pbias)

    in_maps = make_in_maps(x)
    res = run_bass_kernel_spmd(nc, in_maps, core_ids=list(range(NCORES)))
    out = np.stack([res.results[i]["out"] for i in range(NCORES)])
    return out.reshape(B, H, W, C)


if __name__ == "__main__":
    pass



# revision 28
# speedup vs baseline: 1.0182x; 1.0052x over previous
"""Trainium2 Bass kernel for CHMSA (cross-covariance multi-head self-attention
with a ConvNorm qkv stem).

Problem (hardcoded):
  x         [16, 64, 64, 256] f32
  dw_kernel [3, 3, 1, 256]    depthwise 3x3, SAME
  bn_gamma/bn_beta [256]      per-channel affine after dwconv
  pw_kernel [256, 768]        1x1 conv -> qkv
  q_bias/v_bias [256]         qkv bias = concat([q_bias, 0, v_bias])
  scale     [8,1,1]           per-head logit scale, s = exp(min(scale, ln 100))
  proj_w    [256, 256], proj_b [256]

Sharding: pure data-parallel over batch: 16 images / 8 cores = 2 images/core.
No collectives.

Everything upstream of PSUM runs in fp16 (e5m10): fp16's 10-bit mantissa
keeps the end-to-end rel err at ~2.8e-3 (bf16's 7-bit mantissa blows the
2e-2 budget through the weight tensors), while every stationary operand
gets the 2-byte half-cost LDWEIGHTS and x DMA traffic halves. All PSUM
accumulation stays f32.

Per-core dataflow (per image, N = 4096 tokens, C = 256):
  1. x is pre-transposed to channel-major [C, N] and cast to fp16 on the
     HOST (make_in_maps); the xt tile is UNPADDED so each x row-block DMA
     is one contiguous 1KB-per-partition run (a zero-halo layout produced
     128-byte bursts that starved the dwconv at startup).
  2. dwconv: 9 diagonal fp16 matmuls per PSUM tile (channel-major), gamma
     folded into the diagonal weights. SAME padding via per-tap AP
     clipping: edge psum rows/cols simply don't receive out-of-image tap
     contributions. The two channel chunks are emitted as separate halves
     so qk fronts can interleave between them on the PE queue.
  3. qkv: q,k token-major per token-chunk PAIR: ACT square -> GpSimd
     half-fold -> DVE grouped reduce -> rsqrt per half (1/|q|, 1/|k|) ->
     ONE DVE op per chunk scales the whole 512-wide psum tile into the
     fp16 qskb tile (q-half by 1/|q|, k-half by 1/|k|; the per-head scale
     s is folded into the softmax logit gather instead); v channel-major.
  4. attn gram: fp16 [128,128] matmuls accumulated over all 32 token
     chunks. BOTH head-groups accumulate concurrently in one PSUM bank
     (att is [128, 256] = 1KB of the 2KB bank; start_tensor_calc zeroes
     the whole bank, so only the very first matmul starts and only the
     very last stops). Softmax applies s_h during the DVE logit gather;
     32x32 DVE transposes build attn^T (fp16).
  5. out_cm = attn^T-weighted v (channel-major, fp16), proj back to
     token-major (fp16 weights), DMA out. The previous image's C blocks
     interleave into the next image's phase between that stage's qk
     front and gram so the PE never stalls on the previous softmax, and
     the next image's first x row-blocks prefetch during this image's
     late stages (region-level dependency tracking permits it).
"""

import math

import numpy as np
import ml_dtypes

import concourse.bass as bass
import concourse.mybir as mybir
import concourse.tile as tile
from concourse import bacc
from concourse.bass_utils import run_bass_kernel_spmd

F32 = mybir.dt.float32
F32R = mybir.dt.float32r
F16 = mybir.dt.float16
BF16 = mybir.dt.bfloat16
AF = mybir.ActivationFunctionType
ALU = mybir.AluOpType

B, H, W, C = 16, 64, 64, 256
N = H * W              # 4096 tokens per image
HEADS = 8
HD = C // HEADS        # 32
NCORES = 8
IMGS = B // NCORES     # 2 images per core
NCH = C // 128         # 2 channel chunks
LOG_MAX_SCALE = float(np.log(100.0))

# dwconv tap offsets (dh, dw), center first so it can carry start=True with
# full-tile coverage; the ragged edge taps then accumulate.
TAPS = [(0, 0), (-1, -1), (-1, 0), (-1, 1), (0, -1), (0, 1), (1, -1), (1, 0), (1, 1)]

HBLK = 8               # h-rows per dwconv psum tile -> free dim 8*64 = 512
NBLK = N // 128        # 32 token chunks of 128

# ---- engine assignment knobs ----
# (GpSimd cannot read PSUM, so all PSUM evictions live on ACT/DVE.)
VT_EVICT_DVE = False    # DVE queue feeds the qk chain; keep v eviction on ACT


def _build_program(consts, add_qbias, add_pbias, reps=1):
    nc = bacc.Bacc()

    # x arrives channel-major ([IMGS, C, N], transposed + cast to fp16 on
    # the HOST) so the dwconv input tile loads directly with no PE
    # transposes; fp16 halves the x DMA traffic and makes every stationary
    # weight load a 2-byte (half-cost) LDWEIGHTS.
    x_dr = nc.dram_tensor("x", [IMGS, C, N], F16, kind="ExternalInput")
    out_dr = nc.dram_tensor("out", [IMGS, N, C], F32, kind="ExternalOutput")

    diag_dr = nc.inline_tensor(consts["diag"], "cdiag")        # [128, NCH, 9, 128]
    pwqk_dr = nc.inline_tensor(consts["pwqk"], "cpwqk")        # [128, NCH, 512]
    pwv_dr = nc.inline_tensor(consts["pwv"], "cpwv")           # [128, NCH, NCH, 128]
    projw_dr = nc.inline_tensor(consts["projw"], "cprojw")     # [128, NCH, 256] bf16
    beta_dr = nc.inline_tensor(consts["beta"], "cbeta")        # [128, NCH]
    vb_dr = nc.inline_tensor(consts["vb"], "cvb")              # [128, NCH]
    s_host = [float(v) for v in consts["s_host"]]              # python floats
    if add_qbias:
        qb_dr = nc.inline_tensor(consts["qb"], "cqb")          # [128, 256]
    if add_pbias:
        pb_dr = nc.inline_tensor(consts["pb"], "cpb")          # [128, 256]

    use_beta = bool(np.any(consts["beta"]))
    use_vb = bool(np.any(consts["vb"]))

    with tile.TileContext(nc) as tc:
        with (
            tc.tile_pool(name="singles", bufs=1) as singles,
            tc.tile_pool(name="xt", bufs=1) as xt_pool,
            tc.tile_pool(name="img_big", bufs=1) as img_pool,
            tc.tile_pool(name="sq", bufs=4) as sq_pool,
            tc.tile_pool(name="wp", bufs=4) as wp_pool,
            tc.tile_pool(name="small", bufs=3) as small,
            tc.tile_pool(name="ostage", bufs=6) as ostage,
            tc.tile_pool(name="ps_mm", bufs=2, space="PSUM") as ps_mm,
            tc.tile_pool(name="ps_qk", bufs=3, space="PSUM") as ps_qk,
            tc.tile_pool(name="ps_c", bufs=2, space="PSUM") as ps_c,
            tc.tile_pool(name="ps_attn", bufs=1, space="PSUM") as ps_attn,
        ):
            # ---- constants into SBUF ----
            # Spread the big const DMAs across engine DGE queues so the
            # first dwconv tile isn't gated on one serial queue: diag's
            # two channel chunks go to the vector and scalar queues (the
            # sync + gpsimd queues carry the first x tiles).
            diag_sb = singles.tile([128, NCH, 9, 128], F16)
            nc.scalar.dma_start(diag_sb[:, 0], diag_dr[:, 0])
            nc.scalar.dma_start(diag_sb[:, 1], diag_dr[:, 1])
            pwqk_sb = singles.tile([128, NCH, 512], F16)
            nc.scalar.dma_start(pwqk_sb[:], pwqk_dr[:])
            pwv_sb = singles.tile([128, NCH, NCH, 128], F16)
            nc.scalar.dma_start(pwv_sb[:], pwv_dr[:])
            projw_sb = singles.tile([128, NCH, 256], F16)
            nc.scalar.dma_start(projw_sb[:], projw_dr[:])
            beta_sb = singles.tile([128, NCH], F32)
            nc.scalar.dma_start(beta_sb[:], beta_dr[:])
            vb_sb = singles.tile([128, NCH], F32)
            nc.scalar.dma_start(vb_sb[:], vb_dr[:])
            if add_qbias:
                qb_sb = singles.tile([128, 256], F32)
                nc.gpsimd.dma_start(qb_sb[:], qb_dr[:])
            if add_pbias:
                pb_sb = singles.tile([128, 256], F32)
                nc.gpsimd.dma_start(pb_sb[:], pb_dr[:])

            # xt is shared by both images (re-DMA'd per image), UNPADDED:
            # SAME-padding is expressed by clipping each tap's matmul APs
            # instead of a zero halo, so every x DMA is a contiguous
            # 1KB-per-partition run (the padded layout produced 128-byte
            # bursts that starved the dwconv at startup).
            xt_sh = xt_pool.tile([128, NCH, H, W], F16,
                                 tag="xt", name="xt_sh")

            def make_img_state(img):
                st = {}
                st["img"] = img
                st["xt"] = xt_sh
                st["yt"] = img_pool.tile([128, NCH, N], F16, tag="yt",
                                         name=f"yt{img}")
                st["vt"] = img_pool.tile([128, NCH, N], F16, tag="vt",
                                         name=f"vt{img}")
                # one PSUM bank holds BOTH head-groups' grams ([128, 256]
                # = 1KB of the 2KB bank). start_tensor_calc zeroes the
                # whole bank, so only the first matmul into the bank may
                # carry start=True and only the very last stop=True.
                st["att"] = ps_attn.tile([128, 2, 128], F32, tag="att",
                                         name=f"att_{img}")
                # [q-half | k-half] per token chunk, already l2-scaled
                st["qskb"] = img_pool.tile([128, NBLK, 512], F16, tag="qskb",
                                           name=f"qskb{img}")
                return st

            def load_rb(img, rb, preload=False):
                # DMA one 8-row block (512 tokens) of channel-major x into
                # the xt tile; the two channel chunks ride separate DGE
                # queues (sync + gpsimd) so they transfer in parallel.
                # Preloads for the NEXT image ride the scalar queue instead:
                # it is idle after the startup constants, while the sync
                # queue carries the previous image's out stores at exactly
                # that point in the schedule.
                for cch in range(NCH):
                    eng = nc.sync if cch == 0 else nc.gpsimd
                    eng.dma_start(
                        xt_sh[:, cch, 8 * rb:8 * (rb + 1), :].rearrange(
                            "p h w -> p (h w)"),
                        x_dr[img, cch * 128:(cch + 1) * 128,
                             rb * 512:(rb + 1) * 512],
                    )

            def dwconv_block(st, hb, cch):
                # SAME padding via AP clipping: each tap's matmul writes only
                # the psum rows/cols whose shifted input lies inside the
                # image; edge cells simply receive fewer tap contributions
                # (they were zeroed by the center tap's start_tensor_calc).
                # One channel chunk per call so the caller can interleave
                # other PE work between the halves.
                h0 = hb * HBLK
                if True:
                    ysl = st["yt"][:, cch, h0 * W:(h0 + HBLK) * W]
                    yp = ps_mm.tile([128, HBLK, W], F32, tag="mm", name="yp")
                    for i, ti in enumerate(range(9)):
                        dh, dw = TAPS[ti]
                        r0 = max(0, -(h0 + dh))
                        r1 = HBLK + min(0, H - (h0 + HBLK + dh))
                        c0 = max(0, -dw)
                        c1 = W - max(0, dw)
                        nc.tensor.matmul(
                            yp[:, r0:r1, c0:c1],
                            diag_sb[:, cch, ti, :],
                            st["xt"][:, cch, h0 + r0 + dh:h0 + r1 + dh,
                                     c0 + dw:c1 + dw],
                            start=(i == 0),
                            stop=(i == 8),
                            skip_group_check=True,
                        )
                    ypf = yp.rearrange("p h w -> p (h w)")
                    if use_beta:
                        nc.scalar.activation(
                            out=ysl, in_=ypf, func=AF.Identity,
                            bias=beta_sb[:, cch:cch + 1],
                        )
                    else:
                        nc.scalar.copy(ysl, ypf)

            def v_block(st, nb):
                for vc in range(NCH):
                    vp = ps_mm.tile([128, 512], F32, tag="mm", name="vp")
                    for kc in range(NCH):
                        nc.tensor.matmul(
                            vp[:],
                            pwv_sb[:, kc, vc, :],
                            st["yt"][:, kc, nb * 512:(nb + 1) * 512],
                            start=(kc == 0),
                            stop=(kc == NCH - 1),
                        )
                    vsl = st["vt"][:, vc, nb * 512:(nb + 1) * 512]
                    if VT_EVICT_DVE:
                        nc.vector.tensor_scalar(
                            out=vsl, in0=vp[:], scalar1=vb_sb[:, vc:vc + 1],
                            scalar2=None, op0=ALU.add,
                        )
                    elif use_vb:
                        nc.scalar.activation(
                            out=vsl, in_=vp[:], func=AF.Identity,
                            bias=vb_sb[:, vc:vc + 1],
                        )
                    else:
                        nc.scalar.copy(vsl, vp[:])

            def qk_front(st, p):
                # two token chunks t0,t1: qkv matmuls -> squares (ACT) ->
                # grouped reduce (GpSimd) -> per-half rsqrt (1/|q|, 1/|k|)
                # -> ONE DVE op per chunk writes the whole bf16 qskb tile
                # straight from PSUM (q-half scaled by 1/|q|, k-half by
                # 1/|k|). The gram matmuls are emitted later (qk_gram) so
                # other PE work covers this vector-side latency.
                qps = []
                sqs = []
                for j in (0, 1):
                    t = 2 * p + j
                    qp = ps_qk.tile([128, 512], F32, tag="qk", name=f"qp{j}")
                    for kc in range(NCH):
                        nc.tensor.matmul(
                            qp[:],
                            st["yt"][:, kc, t * 128:(t + 1) * 128],
                            pwqk_sb[:, kc, :],
                            start=(kc == 0),
                            stop=(kc == NCH - 1),
                        )
                    if add_qbias:
                        nc.vector.tensor_tensor(
                            out=qp[:, 0:256], in0=qp[:, 0:256],
                            in1=qb_sb[:], op=ALU.add,
                        )
                    sq = sq_pool.tile([128, 512], F32, name="sq")
                    nc.scalar.square(sq[:], qp[:])
                    qps.append(qp)
                    sqs.append(sq)
                sqr = wp_pool.tile([128, 2, 16], F32, tag="sqr", name="sqr")
                for j in (0, 1):
                    # single full-width DVE grouped reduce: one less engine
                    # hop in the front chain than the GpSimd-fold variant,
                    # which matters because the chain latency gates the qp
                    # PSUM bank rotation.
                    nc.vector.tensor_reduce(
                        out=sqr[:, j, :],
                        in_=sqs[j].rearrange("p (g d) -> p g d", d=HD),
                        axis=mybir.AxisListType.X,
                        op=ALU.add,
                    )
                # sqr = [|q|^2 (8 heads) | |k|^2 (8 heads)] per chunk;
                # in-place rsqrt gives exactly the per-half scale vector.
                sqf = sqr.rearrange("p a h -> p (a h)")
                # sqrt on ACT first, reciprocal on DVE second: the final two
                # chain links (recip -> scaled eviction) then sit adjacent on
                # the DVE queue with no cross-engine hop before the op that
                # frees the qp PSUM bank.
                nc.scalar.activation(sqf, sqf, AF.Sqrt)
                nc.vector.reciprocal(sqf, sqf)
                for j in (0, 1):
                    t = 2 * p + j
                    nc.vector.tensor_tensor(
                        out=st["qskb"][:, t, :].rearrange(
                            "p (g d) -> p g d", d=HD),
                        in0=qps[j].rearrange("p (g d) -> p g d", d=HD),
                        in1=sqr[:, j, :].unsqueeze(2).broadcast_to(
                            [128, 16, HD]),
                        op=ALU.mult,
                    )

            def qk_gram(st, p):
                # both head-groups accumulate in the shared att bank;
                # start only zeroes once (whole-bank zero), stop only on
                # the very last matmul into the bank.
                for j in (0, 1):
                    t = 2 * p + j
                    for g in range(2):
                        nc.tensor.matmul(
                            st["att"][:, g, :],
                            st["qskb"][:, t, g * 128:(g + 1) * 128],
                            st["qskb"][:, t, 256 + g * 128:256 + (g + 1) * 128],
                            start=(t == 0 and g == 0),
                            stop=(t == NBLK - 1 and g == 1),
                            skip_group_check=True,
                        )

            def softmax_g(st, g):
                if g == 0:
                    st["at_bd"] = small.tile([128, 2, 128], F16, tag="atbd",
                                             name="at_bd")
                at_bd = st["at_bd"]
                asm = small.tile([128, 32], F32, tag="asm", name="asm")
                for j in range(4):
                    # partition-disjoint gathers split across ACT and DVE so
                    # the four ops pipeline two-wide instead of serially
                    h = 4 * g + j
                    src_blk = st["att"][32 * j:32 * j + 32, g,
                                        32 * j:32 * j + 32]
                    if j % 2 == 0:
                        nc.scalar.mul(asm[32 * j:32 * j + 32, :], src_blk,
                                      s_host[h])
                    else:
                        nc.vector.tensor_scalar(
                            out=asm[32 * j:32 * j + 32, :], in0=src_blk,
                            scalar1=s_host[h], scalar2=None, op0=ALU.mult,
                        )
                mx = small.tile([128, 1], F32, tag="mx", name="mx")
                nc.vector.tensor_reduce(
                    out=mx[:], in_=asm[:], axis=mybir.AxisListType.X,
                    op=ALU.max, negate=True)
                nc.scalar.activation(asm[:], asm[:], AF.Exp, bias=mx[:])
                sm = small.tile([128, 1], F32, tag="sm", name="sm")
                nc.vector.tensor_reduce(
                    out=sm[:], in_=asm[:], axis=mybir.AxisListType.X,
                    op=ALU.add)
                nc.vector.reciprocal(sm[:], sm[:])
                nc.vector.tensor_scalar(
                    out=asm[:], in0=asm[:], scalar1=sm[:], scalar2=None,
                    op0=ALU.mult)
                atf = small.tile([128, 128], F32, tag="atf", name="atf")
                nc.vector.memset(atf[:], 0.0)
                for j in range(4):
                    nc.vector.transpose(
                        atf[32 * j:32 * j + 32, 32 * j:32 * j + 32],
                        asm[32 * j:32 * j + 32, :],
                    )
                nc.vector.tensor_copy(at_bd[:, g, :], atf[:])

            def get_ocm(st):
                img = st["img"]
                if "ocm" not in st:
                    st["ocm"] = img_pool.tile([128, NCH, N], F16, tag="ocm",
                                              name=f"ocm{img}")
                return st["ocm"]

            def c_av(st, nb, g, tail=False):
                # attn^T @ v for one head-group over one 512-token slab; in
                # the trailing loop the tiles borrow the idle ps_qk banks.
                ocm = get_ocm(st)
                op_ = (ps_qk if tail else ps_c).tile(
                    [128, 512], F32, tag="qk" if tail else "cmm",
                    name="op_")
                nc.tensor.matmul(
                    op_[:],
                    st["at_bd"][:, g, :],
                    st["vt"][:, g, nb * 512:(nb + 1) * 512],
                )
                if g == 0:
                    nc.vector.tensor_copy(
                        ocm[:, g, nb * 512:(nb + 1) * 512], op_[:])
                else:
                    nc.scalar.copy(
                        ocm[:, g, nb * 512:(nb + 1) * 512], op_[:])

            def c_proj(st, nb):
                img = st["img"]
                ocm = get_ocm(st)
                for t in range(4 * nb, 4 * nb + 4):
                    pp = ps_c.tile([128, 256], F32, tag="cmm", name="pp")
                    for kc in range(NCH):
                        nc.tensor.matmul(
                            pp[:],
                            ocm[:, kc, t * 128:(t + 1) * 128],
                            projw_sb[:, kc, :],
                            start=(kc == 0),
                            stop=(kc == NCH - 1),
                        )
                    if t % 2 == 0:
                        ot = ostage.tile([128, 2, 256], F32, name="ot")
                        st["ot"] = ot
                    ot = st["ot"]
                    if add_pbias:
                        nc.vector.tensor_tensor(
                            out=ot[:, t % 2, :], in0=pp[:], in1=pb_sb[:],
                            op=ALU.add)
                    elif t % 2 == 0:
                        nc.scalar.copy(ot[:, 0, :], pp[:])
                    else:
                        nc.vector.tensor_copy(ot[:, 1, :], pp[:])
                    if t % 2 == 1:
                        nc.sync.dma_start(
                            out_dr[img, (t - 1) * 128:(t + 1) * 128,
                                   :].rearrange("(g p) c -> p g c", p=128),
                            ot[:],
                        )

            def c_block(st, nb):
                c_av(st, nb, 0)
                c_av(st, nb, 1)
                c_proj(st, nb)

            def phase_A(st, interleave=None, preloaded=False,
                        between_softmax=None):
                # interleave: optional callable(tb) emitting prev-img C blocks
                img = st["img"]

                for s in range(8):
                    h = s - 1
                    # x prefetch runs 2+ row-blocks ahead; when this image's
                    # first blocks were preloaded during the previous image's
                    # phase, only rb3..rb7 remain.
                    if preloaded:
                        if s <= 4:
                            load_rb(img, s + 3)
                    else:
                        if s == 0:
                            load_rb(img, 0)
                            load_rb(img, 1)
                            load_rb(img, 2)
                        elif s <= 5:
                            load_rb(img, s + 2)
                    # the NEXT image's first row-blocks load near the end of
                    # this phase (their xt regions are long since consumed).
                    if s >= 5 and img + 1 < IMGS:
                        load_rb(img + 1, s - 5, preload=True)
                    # PE-queue order spreads the two qk fronts apart (dwconv
                    # halves + v block between them) so each front's qp PSUM
                    # bank has a few microseconds of covering PE work before
                    # the rotation reuses it; each front's vector chain
                    # (square -> fold -> reduce -> rsqrt -> qskb) then never
                    # stalls the PE. The previous image's C block lands
                    # before the grams for the same reason.
                    dwconv_block(st, s, 0)
                    if h >= 0:
                        qk_front(st, 2 * h)
                    if h >= 2:
                        v_block(st, h - 2)
                    dwconv_block(st, s, 1)
                    if h >= 0:
                        qk_front(st, 2 * h + 1)
                    if interleave is not None:
                        interleave(s)
                    if h >= 0:
                        qk_gram(st, 2 * h)
                        qk_gram(st, 2 * h + 1)
                # drain: last qk body (h=7) + remaining v blocks, with the
                # v blocks spread out as PE cover for front(15)'s vector
                # chain so gram(15) never heads an idle PE queue.
                qk_front(st, 14)
                v_block(st, 5)
                qk_front(st, 15)
                v_block(st, 6)
                qk_gram(st, 14)
                v_block(st, 7)
                qk_gram(st, 15)
                # both head-group grams finished accumulating in the shared
                # bank; softmax g0 now, g1 via the hook (the last image's
                # trailing loop inserts its g0 attn@v matmuls in between so
                # the PE covers g1's softmax chain).
                softmax_g(st, 0)
                if between_softmax is not None:
                    between_softmax()
                softmax_g(st, 1)

            import contextlib
            rep_engines = (mybir.EngineType.PE, mybir.EngineType.DVE,
                           mybir.EngineType.Activation, mybir.EngineType.SP,
                           mybir.EngineType.Pool)
            rep_ctx = (tc.For_i(0, reps, 1, hint_engines=rep_engines)
                       if reps > 1 else contextlib.nullcontext())
            with rep_ctx:
                prev = None
                for img in range(IMGS):
                    st = make_img_state(img)
                    if prev is None:
                        phase_A(st)
                    else:
                        pv = prev

                        def emit_c(tb, pv=pv):
                            if tb < 8:
                                c_block(pv, tb)
                        phase_A(st, interleave=emit_c, preloaded=True)
                    prev = st
                for nb in range(8):
                    c_av(prev, nb, 0, tail=True)
                    c_av(prev, nb, 1, tail=True)
                    c_proj(prev, nb)

    nc.finalize()
    return nc


def _prep_consts(dw_kernel, bn_gamma, bn_beta, pw_kernel, q_bias, v_bias,
                 scale, proj_w, proj_b):
    taps_w = np.empty((9, C), np.float32)
    for ti, (dh, dw) in enumerate(TAPS):
        taps_w[ti] = dw_kernel[dh + 1, dw + 1, 0, :] * bn_gamma

    diag = np.zeros((128, NCH, 9, 128), np.float16)
    idx = np.arange(128)
    for cch in range(NCH):
        for ti in range(9):
            diag[idx, cch, ti, idx] = taps_w[ti, cch * 128 + idx].astype(np.float16)

    pwqk = np.empty((128, NCH, 512), np.float16)
    pwv = np.empty((128, NCH, NCH, 128), np.float16)
    for kc in range(NCH):
        pwqk[:, kc, :] = pw_kernel[kc * 128:(kc + 1) * 128, 0:512].astype(np.float16)
        for vc in range(NCH):
            pwv[:, kc, vc, :] = pw_kernel[kc * 128:(kc + 1) * 128,
                                          512 + vc * 128:512 + (vc + 1) * 128].astype(np.float16)

    projw = np.empty((128, NCH, 256), np.float16)
    for kc in range(NCH):
        projw[:, kc, :] = proj_w[kc * 128:(kc + 1) * 128, :].astype(np.float16)

    s_host = np.exp(np.minimum(scale.reshape(HEADS), LOG_MAX_SCALE)).astype(np.float32)

    # tap weights as per-partition columns [128, NCH, 9] for vector-engine
    # dwconv tap offload (scalar_tensor_tensor / activation scale operands)
    kcol = np.empty((128, NCH, 9), np.float32)
    for cch in range(NCH):
        for ti in range(9):
            kcol[:, cch, ti] = taps_w[ti, cch * 128:(cch + 1) * 128]

    consts = {
        "diag": diag,
        "pwqk": pwqk,
        "pwv": pwv,
        "projw": projw,
        "beta": bn_beta.reshape(NCH, 128).T.astype(np.float32).copy(),
        "vb": v_bias.reshape(NCH, 128).T.astype(np.float32).copy(),
        "kcol": kcol,
        "s_host": s_host,
        "qb": np.tile(q_bias[None, :], (128, 1)).astype(np.float32),
        "pb": np.tile(proj_b[None, :], (128, 1)).astype(np.float32),
    }
    return consts


def make_in_maps(x):
    # per-core channel-major fp16 x: [IMGS, C, N]
    xs = np.ascontiguousarray(
        np.asarray(x, np.float32).reshape(NCORES, IMGS, N, C).transpose(
            0, 1, 3, 2).astype(np.float16))
    return [{"x": xs[i]} for i in range(NCORES)]


def kernel(x, dw_kernel, bn_gamma, bn_beta, pw_kernel, q_bias, v_bias, scale,
           proj_w, proj_b):
    consts = _prep_consts(
        np.asarray(dw_kernel, np.float32), np.asarray(bn_gamma, np.float32),
        np.asarray(bn_beta, np.float32), np.asarray(pw_kernel, np.float32),
        np.asarray(q_bias, np.float32), np.asarray(v_bias, np.float32),
        np.asarray(scale, np.float32), np.asarray(proj_w, np.float32),
        np.asarray(proj_b, np.float32))

    add_qbias = bool(np.any(q_bias))
    add_pbias = bool(np.any(proj_b))
    nc = _build_program(consts, add_qbias, add_pbias)

    in_maps = make_in_maps(x)
    res = run_bass_kernel_spmd(nc, in_maps, core_ids=list(range(NCORES)))
    out = np.stack([res.results[i]["out"] for i in range(NCORES)])
    return out.reshape(B, H, W, C)


if __name__ == "__main__":
    pass

